# revision 3
# baseline (speedup 1.0000x reference)
# Trainium2 Bass kernel for nn_Decoder_51582557225714.
# 8-way tensor-parallel single-layer decoder with cross-attention.
#
# Sharding (per core c of 8):
#  - q/k/v/o, cross q/k/v/o: column-shard by head (4 heads = 512 cols per core),
#    o/cwo row-sharded; partial outputs AllReduced.
#  - MLP gate/up column-shard (1376 -> padded 1408 cols), down row-shard, AllReduce.
#  - projector: p_w1 column-shard (1024 cols of PH), p_w2 row-shard, AllReduce.
#  - lm_head vocab-shard (1000 cols per core), gathered on host.
#  - embedding gather + all input sharding/transposition done host-side.
# All activations kept TRANSPOSED ([feature, seq]) on device; fp16 data with
# fp32 PSUM accumulation; rmsnorm folded into weights (ln scale) + column
# rescale (rsqrt); softmax without max-subtraction (scores are O(+-8)).

import math
import numpy as np

import concourse.bass as bass
import concourse.mybir as mybir
import concourse.tile as tile
from concourse import bacc
from concourse.bass_utils import run_bass_kernel_spmd

P = 128
NCORES = 8
B, S, MLEN = 1, 1024, 1024
D, H, DH, FF = 4096, 32, 128, 11008
V, DM, PH = 8000, 1024, 8192
EPS = 1e-6

DKT = D // P            # 32 k-tiles over D
DMKT = DM // P          # 8
HSH = H // NCORES       # 4 heads per core
DSH = HSH * DH          # 512
FFSH = FF // NCORES     # 1376
FFPAD = 1408            # padded to 11*128
FFKT = FFPAD // P       # 11
PHS = PH // NCORES      # 1024
PHKT = PHS // P         # 8
VSH = V // NCORES       # 1000
SKT = S // P            # 8

f32 = mybir.dt.float32
f16 = mybir.dt.float16
AF = mybir.ActivationFunctionType
ALU = mybir.AluOpType

_prog_cache = {}
LAST_RESULTS = None  # BassKernelResults of the most recent run (for harness use)


def _chunks(lo, hi, bank=512):
    """Bank-aligned chunks of [lo, hi) with width <= bank."""
    out = []
    c0 = (lo // bank) * bank
    while c0 < hi:
        a = max(lo, c0)
        b = min(hi, c0 + bank)
        if a < b:
            out.append((a, b))
        c0 += bank
    return out


def _emit_norm(nc, tc, ctxname, hT, ones, scratch_rs, want_q=False, want_t=False):
    """sumsq over partition-tiled hT -> rsqrt(mean+eps) per seq position.
    Returns (rbc [128,S] f32, rbcq or None, rT [128,SKT] f32 or None)."""
    with (
        tc.tile_pool(name=f"{ctxname}_sqp", bufs=3) as sqp,
        tc.tile_pool(name=f"{ctxname}_sps", bufs=1, space="PSUM") as sps,
    ):
        ps = sps.tile([1, S], f32)
        for kt in range(DKT):
            hsq = sqp.tile([P, S], f16, tag="hsq")
            nc.scalar.activation(hsq[:], hT[:, kt, :], AF.Square)
            for c0, c1 in _chunks(0, S):
                nc.tensor.matmul(ps[0:1, c0:c1], ones[:, 0:1], hsq[:, c0:c1],
                                 start=(kt == 0), stop=(kt == DKT - 1))
        row = sqp.tile([1, S], f32, tag="row")
        nc.scalar.activation(row[:], ps[0:1, :], AF.Sqrt, scale=1.0 / D,
                             bias=tc.eps_t[0:1, 0:1])
        rrow = sqp.tile([1, S], f32, tag="rrow")
        nc.vector.reciprocal(rrow[:], row[:])

        rbc = tc.norm_pool.tile([P, S], f32, tag=f"{ctxname}_rbc")
        nc.gpsimd.partition_broadcast(rbc[:], rrow[0:1, :])
        rbcq = None
        if want_q:
            rbcq = tc.norm_pool.tile([P, S], f32, tag=f"{ctxname}_rbcq")
            nc.vector.tensor_scalar_mul(rbcq[:], rbc[:], 1.0 / math.sqrt(DH))
        rT = None
        if want_t:
            nc.sync.dma_start(out=scratch_rs[:], in_=rrow[0:1, :])
            rT = tc.norm_pool.tile([P, SKT], f32, tag=f"{ctxname}_rT")
            nc.sync.dma_start(out=rT[:], in_=scratch_rs.ap().rearrange("(kt p) -> p kt", p=P))
    return rbc, rbcq, rT


def _emit_attention(nc, tc, ctxname, qkT, v_sb, ones, maskT, attn_oT):
    """Causal attention for HSH heads. qkT [128, 2*HSH, S] f16 (q tiles then k
    tiles, already scaled/roped). v_sb [128, SKT, DSH] f16 (seq-partitioned).
    Writes attn_oT [128, HSH, S] f16."""
    for h in range(HSH):
        qTh = qkT[:, h, :]
        kTh = qkT[:, HSH + h, :]
        with (
            tc.tile_pool(name=f"{ctxname}_at{h}", bufs=2) as atp,
            tc.tile_pool(name=f"{ctxname}_aps{h}", bufs=2, space="PSUM") as aps,
            tc.tile_pool(name=f"{ctxname}_apo{h}", bufs=1, space="PSUM") as apo,
        ):
            ps_o = apo.tile([P, S], f32, tag="ps_o")
            ps_cs = apo.tile([1, S], f32, tag="ps_cs")
            for kt in range(SKT):
                n0 = kt * P
                ps_s = aps.tile([P, S], f32, tag="ps_s")
                for c0, c1 in _chunks(n0, S):
                    nc.tensor.matmul(ps_s[:, c0:c1], kTh[:, n0:n0 + P], qTh[:, c0:c1],
                                     start=True, stop=True)
                pT = atp.tile([P, S], f16, tag="pT")
                if n0 > 0:
                    nc.vector.memset(pT[:, 0:n0], 0.0)
                # exp(score - 5): softmax is shift-invariant; keeps exp in
                # fp16 range even for outlier scores (overflow needs >16).
                nc.scalar.activation(pT[:, n0:S], ps_s[:, n0:S], AF.Exp,
                                     bias=tc.nexp_t[:, 0:1])
                nc.vector.tensor_mul(pT[:, n0:n0 + P], pT[:, n0:n0 + P], maskT[:])
                for c0, c1 in _chunks(0, S):
                    nc.tensor.matmul(ps_cs[0:1, c0:c1], ones[:, 0:1], pT[:, c0:c1],
                                     start=(kt == 0), stop=(kt == SKT - 1))
                    nc.tensor.matmul(ps_o[:, c0:c1], v_sb[:, kt, h * DH:(h + 1) * DH],
                                     pT[:, c0:c1], start=(kt == 0), stop=(kt == SKT - 1))
            rrow = atp.tile([1, S], f32, tag="rrow")
            nc.vector.reciprocal(rrow[:], ps_cs[0:1, :])
            rbc = atp.tile([P, S], f32, tag="rbc")
            nc.gpsimd.partition_broadcast(rbc[:], rrow[0:1, :])
            nc.vector.tensor_mul(attn_oT[:, h, :], ps_o[:], rbc[:])


def _emit_proj_stream(nc, tc, ctxname, w_dram, nmt, nkt, rhs_fn, evict_fn,
                      mt_width=P):
    """Generic 'weight-stationary' projection: out[mt] = sum_kt w[:,kt,mslice].T @ rhs[kt].
    w_dram: [128, nkt, nmt*mt_width] f16. rhs_fn(kt, c0, c1) -> AP [128, c1-c0].
    evict_fn(mt, psum_tile) consumes psum [mw, S]."""
    with (
        tc.tile_pool(name=f"{ctxname}_wp", bufs=3) as wp,
        tc.tile_pool(name=f"{ctxname}_pp", bufs=2, space="PSUM") as pp,
    ):
        total = w_dram.shape[2]
        for mt in range(nmt):
            m0 = mt * mt_width
            mw = min(mt_width, total - m0)
            wt = wp.tile([P, nkt, mt_width], f16, tag="wt")
            nc.sync.dma_start(out=wt[:, :, 0:mw], in_=w_dram[:, :, m0:m0 + mw])
            ps = pp.tile([P, S], f32, tag="ps")
            for c0, c1 in _chunks(0, S):
                for kt in range(nkt):
                    nc.tensor.matmul(ps[0:mw, c0:c1], wt[:, kt, 0:mw],
                                     rhs_fn(kt, c0, c1),
                                     start=(kt == 0), stop=(kt == nkt - 1))
            evict_fn(mt, ps, mw)


def _build_program():
    nc = bacc.Bacc("TRN2", target_bir_lowering=False, debug=False,
                   enable_asserts=True, num_devices=NCORES)

    # ---- I/O declarations (per core) ----
    def din(name, shape, dt=f16):
        return nc.dram_tensor(name, shape, dt, kind="ExternalInput")

    hT0_d = din("hT0", [P, DKT, S])
    memT_d = din("memT", [P, DMKT, MLEN])
    pw1_d = din("pw1", [P, DMKT, PHS])
    pw2_d = din("pw2", [P, PHKT, D])
    pb1_d = din("pb1", [P, PHKT], f32)
    pb2_d = din("pb2", [P, DKT], f32)          # p_b2 / 8
    wqk_d = din("wqk", [P, DKT, 2 * DSH])
    wv_d = din("wv", [P, DKT, DSH])
    wo_d = din("wo", [P, DSH // P, D])
    cwqk_d = din("cwqk", [P, DKT, 2 * DSH])
    cwv_d = din("cwv", [P, DKT, DSH])
    cwo_d = din("cwo", [P, DSH // P, D])
    wgu_d = din("wgu", [P, DKT, 2 * FFPAD])
    wd_d = din("wd", [P, FFKT, D])
    lmh_d = din("lmh", [P, DKT, VSH])
    cosT_d = din("cosT", [P, S])
    sinT_d = din("sinT", [P, S])
    rotM_d = din("rotM", [P, P])
    maskT_d = din("maskT", [P, P])

    logits_d = nc.dram_tensor("logitsT", [VSH, S], f32, kind="ExternalOutput")

    # collective bounce buffers
    mem_par = nc.dram_tensor("mem_par", [P, DKT, MLEN], f16)
    mem_red = nc.dram_tensor("mem_red", [P, DKT, MLEN], f16, addr_space="Shared")
    blk_par = [nc.dram_tensor(f"blk_par{i}", [P, DKT, S], f16) for i in range(3)]
    blk_red = [nc.dram_tensor(f"blk_red{i}", [P, DKT, S], f16, addr_space="Shared")
               for i in range(3)]
    scratch_rs = [nc.dram_tensor(f"rs_scratch{i}", [S], f32) for i in range(2)]

    rg = [list(range(NCORES))]

    with tile.TileContext(nc) as tc:
        with (
            tc.tile_pool(name="persist", bufs=1) as persist,
            tc.tile_pool(name="normp", bufs=1) as norm_pool,
        ):
            tc.norm_pool = norm_pool
            hT = persist.tile([P, DKT, S], f16)
            nc.sync.dma_start(out=hT[:], in_=hT0_d[:])
            cosT = persist.tile([P, S], f16)
            sinT = persist.tile([P, S], f16)
            rotM = persist.tile([P, P], f16)
            maskT = persist.tile([P, P], f16)
            ones = persist.tile([P, 1], f16)
            nc.sync.dma_start(out=cosT[:], in_=cosT_d[:])
            nc.sync.dma_start(out=sinT[:], in_=sinT_d[:])
            nc.sync.dma_start(out=rotM[:], in_=rotM_d[:])
            nc.sync.dma_start(out=maskT[:], in_=maskT_d[:])
            nc.vector.memset(ones[:], 1.0)
            eps_t = persist.tile([1, 1], f32)
            nc.vector.memset(eps_t[:], EPS)
            tc.eps_t = eps_t
            nexp_t = persist.tile([P, 1], f32)
            nc.vector.memset(nexp_t[:], -5.0)
            tc.nexp_t = nexp_t

            # ================= projector =================
            with (
                tc.tile_pool(name="proj", bufs=1) as projp,
                tc.tile_pool(name="proj_ev", bufs=3) as projev,
            ):
                memT_sb = projp.tile([P, DMKT, MLEN], f16)
                nc.sync.dma_start(out=memT_sb[:], in_=memT_d[:])
                pb1_sb = projp.tile([P, PHKT], f32)
                pb2_sb = projp.tile([P, DKT], f32)
                nc.sync.dma_start(out=pb1_sb[:], in_=pb1_d[:])
                nc.sync.dma_start(out=pb2_sb[:], in_=pb2_d[:])
                gT = projp.tile([P, PHKT, MLEN], f16)

                def ev_g(mt, ps, mw):
                    nc.scalar.activation(gT[:, mt, :], ps[:], AF.Gelu,
                                         bias=pb1_sb[:, mt:mt + 1])
                _emit_proj_stream(nc, tc, "pj1", pw1_d, PHKT, DMKT,
                                  lambda kt, c0, c1: memT_sb[:, kt, c0:c1], ev_g)

                def ev_m(mt, ps, mw):
                    t = projev.tile([P, S], f16, tag="mev")
                    nc.scalar.activation(t[:], ps[:], AF.Identity,
                                         bias=pb2_sb[:, mt:mt + 1])
                    nc.sync.dma_start(out=mem_par[:, mt, :], in_=t[:])
                _emit_proj_stream(nc, tc, "pj2", pw2_d, DKT, PHKT,
                                  lambda kt, c0, c1: gT[:, kt, c0:c1], ev_m)

                nc.gpsimd.collective_compute(
                    "AllReduce", ALU.add, ins=[mem_par[:]], outs=[mem_red[:]],
                    replica_groups=rg)

            # ============ attention block helper ============
            def attention_block(idx, is_self):
                nm = f"b{idx}"
                rbc, rbcq, rT = _emit_norm(nc, tc, nm, hT, ones, scratch_rs[idx % 2],
                                           want_q=True, want_t=is_self)
                with tc.tile_pool(name=f"{nm}_act", bufs=1) as actp:
                    qkT = actp.tile([P, 2 * HSH, S], f16)
                    v_sb = actp.tile([P, SKT, DSH], f16)

                    if is_self:
                        def ev_qk(mt, ps, mw):
                            nc.scalar.activation(qkT[:, mt, :], ps[:], AF.Copy)
                        _emit_proj_stream(nc, tc, f"{nm}qk", wqk_d, 2 * HSH, DKT,
                                          lambda kt, c0, c1: hT[:, kt, c0:c1], ev_qk)
                    else:
                        def ev_q(mt, ps, mw):
                            nc.scalar.activation(qkT[:, mt, :], ps[:], AF.Copy)
                        _emit_proj_stream(
                            nc, tc, f"{nm}q", cwqk_d.ap()[:, :, 0:DSH], HSH, DKT,
                            lambda kt, c0, c1: hT[:, kt, c0:c1], ev_q)

                        with tc.tile_pool(name=f"{nm}_ms", bufs=3) as mstrp:
                            def rhs_mem(kt, c0, c1):
                                t_ = mstrp.tile([P, 512], f16, tag="ms")
                                nc.sync.dma_start(out=t_[:, 0:c1 - c0],
                                                  in_=mem_red[:, kt, c0:c1])
                                return t_[:, 0:c1 - c0]

                            def ev_k(mt, ps, mw):
                                nc.scalar.activation(qkT[:, HSH + mt, :], ps[:],
                                                     AF.Copy)
                            _emit_proj_stream(
                                nc, tc, f"{nm}k", cwqk_d.ap()[:, :, DSH:2 * DSH],
                                HSH, DKT, rhs_mem, ev_k)

                    # v projection: lhsT = (hT | memT) seq slices, rhs = wv tiles
                    wv_src = wv_d if is_self else cwv_d
                    with (
                        tc.tile_pool(name=f"{nm}_vw", bufs=3) as vwp,
                        tc.tile_pool(name=f"{nm}_vps", bufs=1, space="PSUM") as vps,
                    ):
                        for half in range(2):
                            pss = [vps.tile([P, DSH], f32, tag=f"psv{i}", name=f"psv_{half}_{i}")
                                   for i in range(4)]
                            for kt in range(DKT):
                                wvt = vwp.tile([P, DSH], f16, tag="wvt")
                                nc.sync.dma_start(out=wvt[:], in_=wv_src[:, kt, :])
                                if is_self:
                                    src_t = hT[:, kt, :]
                                else:
                                    mm_t = vwp.tile([P, MLEN], f16, tag="vmem")
                                    nc.sync.dma_start(out=mm_t[:],
                                                      in_=mem_red[:, kt, :])
                                    src_t = mm_t[:]
                                for i in range(4):
                                    mt = half * 4 + i
                                    nc.tensor.matmul(
                                        pss[i][:], src_t[:, mt * P:(mt + 1) * P],
                                        wvt[:], start=(kt == 0), stop=(kt == DKT - 1))
                            for i in range(4):
                                mt = half * 4 + i
                                if is_self:
                                    nc.scalar.activation(v_sb[:, mt, :], pss[i][:],
                                                         AF.Copy, scale=rT[:, mt:mt + 1])
                                else:
                                    nc.scalar.activation(v_sb[:, mt, :], pss[i][:],
                                                         AF.Copy)

                    # rope (self only, via rotation-matrix matmul) + q/k scaling
                    with (
                        tc.tile_pool(name=f"{nm}_rp", bufs=2) as rp,
                        tc.tile_pool(name=f"{nm}_rps", bufs=2, space="PSUM") as rps,
                    ):
                        for t in range(2 * HSH):
                            is_q = t < HSH
                            sc = rbcq if is_q else rbc
                            if is_self:
                                psr = rps.tile([P, S], f32, tag="psr")
                                for c0, c1 in _chunks(0, S):
                                    nc.tensor.matmul(psr[:, c0:c1], rotM[:],
                                                     qkT[:, t, c0:c1],
                                                     start=True, stop=True)
                                t2 = rp.tile([P, S], f16, tag="t2")
                                nc.vector.tensor_mul(t2[:], psr[:], sinT[:])
                                t3 = rp.tile([P, S], f16, tag="t3")
                                nc.vector.tensor_mul(t3[:], qkT[:, t, :], cosT[:])
                                nc.vector.tensor_add(t2[:], t2[:], t3[:])
                                nc.vector.tensor_mul(qkT[:, t, :], t2[:], sc[:])
                            else:
                                if is_q:
                                    nc.vector.tensor_mul(qkT[:, t, :], qkT[:, t, :],
                                                         sc[:])
                    attn_oT = actp.tile([P, HSH, S], f16)
                    _emit_attention(nc, tc, nm, qkT, v_sb, ones, maskT, attn_oT)

                    # o-projection + residual/8 -> AllReduce -> hT
                    wo_src = wo_d if is_self else cwo_d
                    with tc.tile_pool(name=f"{nm}_oev", bufs=3) as oev:
                        def ev_o(mt, ps, mw):
                            t_ = oev.tile([P, S], f16, tag="oev")
                            nc.vector.scalar_tensor_tensor(
                                t_[:], hT[:, mt, :], 1.0 / NCORES, ps[:],
                                ALU.mult, ALU.add)
                            nc.sync.dma_start(out=blk_par[idx][:, mt, :], in_=t_[:])
                        _emit_proj_stream(nc, tc, f"{nm}o", wo_d if is_self else cwo_d,
                                          DKT, DSH // P,
                                          lambda kt, c0, c1: attn_oT[:, kt, c0:c1],
                                          ev_o)
                    nc.gpsimd.collective_compute(
                        "AllReduce", ALU.add, ins=[blk_par[idx][:]],
                        outs=[blk_red[idx][:]], replica_groups=rg)
                    nc.sync.dma_start(out=hT[:], in_=blk_red[idx][:])

            attention_block(0, True)
            attention_block(1, False)

            # ================= MLP =================
            rbc2, _, _ = _emit_norm(nc, tc, "mlp", hT, ones, scratch_rs[0])
            with tc.tile_pool(name="mlp_act", bufs=1) as mlpp:
                guT = mlpp.tile([P, 2 * FFKT, S], f16)

                def ev_gu(mt, ps, mw):
                    nc.scalar.activation(guT[:, mt, :], ps[:], AF.Copy)
                _emit_proj_stream(nc, tc, "mgu", wgu_d, 2 * FFKT, DKT,
                                  lambda kt, c0, c1: hT[:, kt, c0:c1], ev_gu)

                with tc.tile_pool(name="mlp_sw", bufs=2) as swp:
                    for ft in range(FFKT):
                        gs = swp.tile([P, S], f16, tag="gs")
                        nc.vector.tensor_mul(gs[:], guT[:, ft, :], rbc2[:])
                        sg = swp.tile([P, S], f16, tag="sg")
                        nc.scalar.activation(sg[:], gs[:], AF.Silu)
                        us = swp.tile([P, S], f16, tag="us")
                        nc.vector.tensor_mul(us[:], guT[:, FFKT + ft, :], rbc2[:])
                        nc.vector.tensor_mul(guT[:, ft, :], sg[:], us[:])

                with tc.tile_pool(name="mlp_oev", bufs=3) as moev:
                    def ev_d(mt, ps, mw):
                        t_ = moev.tile([P, S], f16, tag="dev")
                        nc.vector.scalar_tensor_tensor(
                            t_[:], hT[:, mt, :], 1.0 / NCORES, ps[:],
                            ALU.mult, ALU.add)
                        nc.sync.dma_start(out=blk_par[2][:, mt, :], in_=t_[:])
                    _emit_proj_stream(nc, tc, "md", wd_d, DKT, FFKT,
                                      lambda kt, c0, c1: guT[:, kt, c0:c1], ev_d)
                nc.gpsimd.collective_compute(
                    "AllReduce", ALU.add, ins=[blk_par[2][:]],
                    outs=[blk_red[2][:]], replica_groups=rg)
                nc.sync.dma_start(out=hT[:], in_=blk_red[2][:])

            # ================= lm head =================
            rbc3, _, _ = _emit_norm(nc, tc, "lmh", hT, ones, scratch_rs[1])
            with tc.tile_pool(name="lmh_ev", bufs=3) as lev:
                def ev_l(mt, ps, mw):
                    t_ = lev.tile([P, S], f32, tag="lev")
                    nc.vector.tensor_mul(t_[0:mw, :], ps[0:mw, :], rbc3[0:mw, :])
                    nc.sync.dma_start(out=logits_d[mt * P:mt * P + mw, :],
                                      in_=t_[0:mw, :])
                _emit_proj_stream(nc, tc, "lh", lmh_d, (VSH + P - 1) // P, DKT,
                                  lambda kt, c0, c1: hT[:, kt, c0:c1], ev_l)

    nc.compile()
    return nc


def _part(x, kt):
    """[R, C] -> [128, R//128, C] with row = kt_idx*128 + p."""
    R, C = x.shape
    return np.ascontiguousarray(x.reshape(kt, P, C).transpose(1, 0, 2))


def kernel(**inputs):
    inp = {k: np.asarray(v) for k, v in inputs.items()}
    ids = inp["input_ids"].astype(np.int64)[0]          # [S]
    memory = inp["memory"].astype(np.float32)[0]        # [MLEN, DM]
    f = np.float32

    ln1 = inp["ln1"].astype(f)
    lnc = inp["lnc"].astype(f)
    ln2 = inp["ln2"].astype(f)
    lnf = inp["lnf"].astype(f)

    h0 = inp["embed"].astype(f)[ids]                    # [S, D]
    hT0 = _part(h0.T.astype(np.float16), DKT)           # [128, 32, S]
    memT = _part(memory.T.astype(np.float16), DMKT)     # [128, 8, MLEN]

    # RoPE tables (transposed layout [DH, S])
    inv = 1.0 / (10000.0 ** (np.arange(0, DH, 2, dtype=f) / DH))
    t = np.arange(S, dtype=f)
    freqs = np.outer(t, inv)                            # [S, DH//2]
    emb = np.concatenate([freqs, freqs], axis=1)        # [S, DH]
    cosT = np.cos(emb).T.astype(np.float16)             # [DH, S]
    sinT = np.sin(emb).T.astype(np.float16)
    rotM = np.zeros((P, P), dtype=np.float16)           # rotM[k,d]: rot_half
    rotM[np.arange(64) + 64, np.arange(64)] = -1.0      # out[d<64] = -in[d+64]
    rotM[np.arange(64), np.arange(64) + 64] = 1.0       # out[d>=64] = in[d-64]
    maskT = np.triu(np.ones((P, P), dtype=np.float16))  # [key p, query col]

    wq = inp["wq"].astype(f) * ln1[:, None]
    wk = inp["wk"].astype(f) * ln1[:, None]
    wv = inp["wv"].astype(f) * ln1[:, None]
    cwq = inp["cwq"].astype(f) * lnc[:, None]
    cwk = inp["cwk"].astype(f)
    cwv = inp["cwv"].astype(f)
    wg = inp["wg"].astype(f) * ln2[:, None]
    wu = inp["wu"].astype(f) * ln2[:, None]
    lmh = inp["lm_head"].astype(f) * lnf[:, None]
    wo = inp["wo"].astype(f)
    cwo = inp["cwo"].astype(f)
    wd = inp["wd"].astype(f)
    pw1 = inp["p_w1"].astype(f)
    pw2 = inp["p_w2"].astype(f)
    pb1 = inp["p_b1"].astype(f)
    pb2 = inp["p_b2"].astype(f)

    h16 = np.float16
    in_maps = []
    for c in range(NCORES):
        ds = slice(c * DSH, (c + 1) * DSH)
        ffs = slice(c * FFSH, (c + 1) * FFSH)
        phs = slice(c * PHS, (c + 1) * PHS)
        vs = slice(c * VSH, (c + 1) * VSH)

        wgu_c = np.zeros((D, 2 * FFPAD), dtype=h16)
        wgu_c[:, 0:FFSH] = wg[:, ffs].astype(h16)
        wgu_c[:, FFPAD:FFPAD + FFSH] = wu[:, ffs].astype(h16)
        wd_c = np.zeros((FFPAD, D), dtype=h16)
        wd_c[0:FFSH] = wd[ffs, :].astype(h16)

        m = {
            "hT0": hT0, "memT": memT,
            "pw1": _part(pw1[:, phs].astype(h16), DMKT),
            "pw2": _part(pw2[phs, :].astype(h16), PHKT),
            "pb1": np.ascontiguousarray(pb1[phs].reshape(PHKT, P).T.astype(f)),
            "pb2": np.ascontiguousarray((pb2 / NCORES).reshape(DKT, P).T.astype(f)),
            "wqk": _part(np.concatenate([wq[:, ds], wk[:, ds]], axis=1).astype(h16), DKT),
            "wv": _part(wv[:, ds].astype(h16), DKT),
            "wo": _part(wo[ds, :].astype(h16), DSH // P),
            "cwqk": _part(np.concatenate([cwq[:, ds], cwk[:, ds]], axis=1).astype(h16), DKT),
            "cwv": _part(cwv[:, ds].astype(h16), DKT),
            "cwo": _part(cwo[ds, :].astype(h16), DSH // P),
            "wgu": _part(wgu_c, DKT),
            "wd": _part(wd_c, FFKT),
            "lmh": _part(lmh[:, vs].astype(h16), DKT),
            "cosT": cosT, "sinT": sinT, "rotM": rotM, "maskT": maskT,
        }
        in_maps.append(m)

    if "nc" not in _prog_cache:
        _prog_cache["nc"] = _build_program()
    nc = _prog_cache["nc"]

    res = run_bass_kernel_spmd(nc, in_maps, list(range(NCORES)))
    global LAST_RESULTS
    LAST_RESULTS = res
    logits = np.concatenate([r["logitsT"].T for r in res.results], axis=1)
    return logits.reshape(B, S, V).astype(np.float32)


if __name__ == "__main__":
    # quick build check
    nc = _build_program()
    print("program built ok")



# revision 15
# speedup vs baseline: 1.1314x; 1.1314x over previous
# Trainium2 Bass kernel for nn_Decoder_51582557225714.
# 8-way tensor-parallel single-layer decoder with cross-attention.
#
# Sharding (per core c of 8):
#  - q/k/v/o, cross q/k/v/o: column-shard by head (4 heads = 512 cols per core),
#    o/cwo row-sharded; partial outputs AllReduced.
#  - MLP gate/up column-shard (1376 -> padded 1408 cols), down row-shard, AllReduce.
#  - projector: p_w1 column-shard (1024 cols of PH), p_w2 row-shard, AllReduce.
#  - lm_head vocab-shard (1000 cols per core), gathered on host.
#  - embedding gather + all input sharding/transposition done host-side.
#
# v2 schedule (keeps TensorE dense + overlaps every collective):
#  - projector runs first (its matmuls warm the PE while hT0 uploads);
#    mem AllReduce rides under block0's projections.
#  - projected memory is AllReduced once and then kept resident in SBUF, so
#    cross-attention k/v projections stream at full rate (no HBM re-reads).
#  - cross k/v projections are emitted right after block0's o-projection so
#    they execute during block0's residual AllReduce.
#  - every residual AllReduce is split into two S-halves; producers evict
#    chunk-major so the first half reduces while the second is computed, and
#    consumers (norm + next projection) start on half 0 while half 1 reduces.
#  - attention is causally trimmed (no matmuls/exp on the all-zero triangle)
#    and processed chunk-major with per-kt probability tiles.
#  - swiglu is fused into the gate/up evictions (wgu tiles interleaved g,u).
# All activations kept TRANSPOSED ([feature, seq]) on device; fp16 data with
# fp32 PSUM accumulation; rmsnorm folded into weights (ln scale) + column
# rescale (rsqrt); softmax without max-subtraction, exp(score-5).

import math
import numpy as np

import concourse.bass as bass
import concourse.mybir as mybir
import concourse.tile as tile
from concourse import bacc
from concourse.bass_utils import run_bass_kernel_spmd

P = 128
NCORES = 8
B, S, MLEN = 1, 1024, 1024
D, H, DH, FF = 4096, 32, 128, 11008
V, DM, PH = 8000, 1024, 8192
EPS = 1e-6

DKT = D // P            # 32 k-tiles over D
DMKT = DM // P          # 8
HSH = H // NCORES       # 4 heads per core
DSH = HSH * DH          # 512
FFSH = FF // NCORES     # 1376
FFPAD = 1408            # padded to 11*128
FFKT = FFPAD // P       # 11
PHS = PH // NCORES      # 1024
PHKT = PHS // P         # 8
VSH = V // NCORES       # 1000
SKT = S // P            # 8
CH = ((0, 512), (512, 1024))

f32 = mybir.dt.float32
f16 = mybir.dt.float16
AF = mybir.ActivationFunctionType
ALU = mybir.AluOpType

_prog_cache = {}
LAST_RESULTS = None  # BassKernelResults of the most recent run (for harness use)


def _build_program():
    nc = bacc.Bacc("TRN2", target_bir_lowering=False, debug=False,
                   enable_asserts=True, num_devices=NCORES)

    # ---- I/O declarations (per core) ----
    def din(name, shape, dt=f16):
        return nc.dram_tensor(name, shape, dt, kind="ExternalInput")

    hT0_d = din("hT0", [P, DKT, S])
    memT_d = din("memT", [P, DMKT, MLEN])
    pw1_d = din("pw1", [P, DMKT, PHS])
    pw2_d = din("pw2", [P, PHKT, D])
    pb1_d = din("pb1", [P, PHKT], f32)
    pb2_d = din("pb2", [P, DKT], f32)          # p_b2 / 8
    wqk_d = din("wqk", [P, DKT, 2 * DSH])
    wv_d = din("wv", [P, DKT, DSH])
    wo_d = din("wo", [P, DSH // P, D])
    cwqk_d = din("cwqk", [P, DKT, 2 * DSH])
    cwv_d = din("cwv", [P, DKT, DSH])
    cwo_d = din("cwo", [P, DSH // P, D])
    wgu_d = din("wgu", [P, DKT, 2 * FFPAD])    # tiles interleaved g0,u0,g1,u1,...
    wd_d = din("wd", [P, FFKT, D])
    lmh_d = din("lmh", [P, DKT, VSH])
    cosT_d = din("cosT", [P, S])
    sinT_d = din("sinT", [P, S])
    rotM_d = din("rotM", [P, P])
    maskT_d = din("maskT", [P, P])

    logits_d = nc.dram_tensor("logitsT", [VSH, S], f32, kind="ExternalOutput")

    # collective bounce buffers
    mem_par = nc.dram_tensor("mem_par", [P, DKT, MLEN], f16)
    mem_red = nc.dram_tensor("mem_red", [P, DKT, MLEN], f16, addr_space="Shared")
    blk_par = [[nc.dram_tensor(f"blk_par{i}_{c}", [P, DKT, 512], f16)
                for c in range(2)] for i in range(3)]
    blk_red = [[nc.dram_tensor(f"blk_red{i}_{c}", [P, DKT, 512], f16,
                               addr_space="Shared")
                for c in range(2)] for i in range(3)]
    scratch_rs = nc.dram_tensor("rs_scratch", [S], f32)

    rg = [list(range(NCORES))]

    with tile.TileContext(nc) as tc:
        with (
            tc.tile_pool(name="persist", bufs=1) as persist,
        ):
            hT = persist.tile([P, DKT, S], f16)
            for q in range(4):
                nc.sync.dma_start(out=hT[:, 8 * q:8 * q + 8, :],
                                  in_=hT0_d[:, 8 * q:8 * q + 8, :])
            cosT = persist.tile([P, S], f16)
            sinT = persist.tile([P, S], f16)
            rotM = persist.tile([P, P], f16)
            maskT = persist.tile([P, P], f16)
            ones = persist.tile([P, 1], f16)
            nc.sync.dma_start(out=cosT[:], in_=cosT_d[:])
            nc.sync.dma_start(out=sinT[:], in_=sinT_d[:])
            nc.sync.dma_start(out=rotM[:], in_=rotM_d[:])
            nc.sync.dma_start(out=maskT[:], in_=maskT_d[:])
            nc.vector.memset(ones[:], 1.0)
            eps_t = persist.tile([1, 1], f32)
            nc.vector.memset(eps_t[:], EPS)
            nexp_t = persist.tile([P, 1], f32)
            nc.vector.memset(nexp_t[:], -5.0)

            # ---------- shared emitters ----------
            def emit_norm(nm, pool, c0, c1, want_q=False, want_t=False):
                """rsqrt(mean(h^2)+eps) over cols [c0,c1). Returns
                (rbc [P,w] f32, rbcq or None, rT [P,SKT] f32 or None)."""
                w = c1 - c0
                with (
                    tc.tile_pool(name=f"{nm}_sq", bufs=3) as sqp,
                    tc.tile_pool(name=f"{nm}_sp", bufs=1, space="PSUM") as sps,
                ):
                    ps = sps.tile([1, w], f32)
                    for kt in range(DKT):
                        hsq = sqp.tile([P, w], f16, tag="hsq")
                        src = hT[:, kt, c0:c1]
                        # split the squares across ACT and DVE
                        if kt % 2 == 0:
                            nc.scalar.activation(hsq[:], src, AF.Square)
                        else:
                            nc.vector.tensor_mul(hsq[:], src, src)
                        # one psum bank (512 f32) per matmul
                        for s0 in range(0, w, 512):
                            s1 = min(w, s0 + 512)
                            nc.tensor.matmul(ps[0:1, s0:s1], ones[:, 0:1],
                                             hsq[:, s0:s1],
                                             start=(kt == 0),
                                             stop=(kt == DKT - 1))
                    row = sqp.tile([1, w], f32, tag="row")
                    nc.scalar.activation(row[:], ps[0:1, :], AF.Sqrt,
                                         scale=1.0 / D, bias=eps_t[0:1, 0:1])
                    rrow = sqp.tile([1, w], f32, tag="rrow")
                    nc.vector.reciprocal(rrow[:], row[:])

                    rbc = pool.tile([P, w], f32, tag=f"{nm}_rbc")
                    nc.gpsimd.partition_broadcast(rbc[:], rrow[0:1, :])
                    rbcq = None
                    if want_q:
                        rbcq = pool.tile([P, w], f32, tag=f"{nm}_rbcq")
                        nc.vector.tensor_scalar_mul(rbcq[:], rbc[:],
                                                    1.0 / math.sqrt(DH))
                    rT = None
                    if want_t:
                        nc.sync.dma_start(out=scratch_rs[:], in_=rrow[0:1, :])
                        rT = pool.tile([P, SKT], f32, tag=f"{nm}_rT")
                        nc.sync.dma_start(
                            out=rT[:],
                            in_=scratch_rs.ap().rearrange("(kt p) -> p kt", p=P))
                return rbc, rbcq, rT

            def emit_attention(nm, qkT, v_sb, attn_o):
                """Causally-trimmed attention, chunk-major over q columns.
                qkT [P, 2*HSH, S] (q tiles 0..HSH-1 scaled, k tiles HSH..).
                v_sb [P, SKT, DSH]. attn_o [P, HSH, S] f16."""
                with (
                    tc.tile_pool(name=f"{nm}_at", bufs=1) as atp,
                    tc.tile_pool(name=f"{nm}_ps", bufs=2, space="PSUM") as psp,
                    tc.tile_pool(name=f"{nm}_po", bufs=2, space="PSUM") as pop,
                    tc.tile_pool(name=f"{nm}_pc", bufs=2, space="PSUM") as pcp,
                ):
                    for ci, (c0, c1) in enumerate(CH):
                        kt_last = c1 // P - 1
                        for h in range(HSH):
                            ps_o = pop.tile([P, 512], f32, tag="ps_o")
                            ps_cs = pcp.tile([1, 512], f32, tag="ps_cs")
                            for kt in range(SKT):
                                n0 = kt * P
                                a0 = max(c0, n0)
                                if a0 >= c1:
                                    continue
                                w = c1 - a0
                                ps_s = psp.tile([P, 512], f32, tag="ps_s")
                                nc.tensor.matmul(
                                    ps_s[:, 0:w], qkT[:, HSH + h, n0:n0 + P],
                                    qkT[:, h, a0:c1], start=True, stop=True)
                                pt = atp.tile([P, w], f16, tag=f"pt{ci}_{kt}")
                                nc.scalar.activation(pt[:], ps_s[:, 0:w], AF.Exp,
                                                     bias=nexp_t[:, 0:1])
                                if a0 == n0:
                                    nc.vector.tensor_mul(pt[:, 0:P], pt[:, 0:P],
                                                         maskT[:])
                                st = (kt == 0)
                                sp = (kt == kt_last)
                                nc.tensor.matmul(ps_cs[0:1, a0 - c0:c1 - c0],
                                                 ones[:, 0:1], pt[:],
                                                 start=st, stop=sp)
                                nc.tensor.matmul(
                                    ps_o[:, a0 - c0:c1 - c0],
                                    v_sb[:, kt, h * DH:(h + 1) * DH], pt[:],
                                    start=st, stop=sp)
                            rrow = atp.tile([1, 512], f32, tag="rrow")
                            nc.vector.reciprocal(rrow[:], ps_cs[0:1, :])
                            rbc = atp.tile([P, 512], f32, tag="rbc")
                            nc.gpsimd.partition_broadcast(rbc[:], rrow[0:1, :])
                            nc.vector.tensor_mul(attn_o[:, h, c0:c1], ps_o[:],
                                                 rbc[:])

            def stream_proj(nm, w_dram, nmt, nkt, rhs_fn, evict_fn,
                            chunks=CH, wbufs=2, total=None):
                """Chunk-major weight-streaming projection.
                out[mt, c0:c1] = sum_kt w[:, kt, mslice].T @ rhs(kt, c0, c1).
                evict_fn(c0, c1, mt, ps, mw) consumes psum [mw, c1-c0]."""
                if total is None:
                    total = nmt * P
                with (
                    tc.tile_pool(name=f"{nm}_w", bufs=wbufs) as wp,
                    tc.tile_pool(name=f"{nm}_p", bufs=2, space="PSUM") as pp,
                ):
                    for c0, c1 in chunks:
                        for mt in range(nmt):
                            m0 = mt * P
                            mw = min(P, total - m0)
                            wt = wp.tile([P, nkt, P], f16, tag="wt")
                            nc.sync.dma_start(out=wt[:, :, 0:mw],
                                              in_=w_dram[:, :, m0:m0 + mw])
                            ps = pp.tile([P, 512], f32, tag="ps")
                            for kt in range(nkt):
                                nc.tensor.matmul(ps[0:mw, 0:c1 - c0],
                                                 wt[:, kt, 0:mw],
                                                 rhs_fn(kt, c0, c1),
                                                 start=(kt == 0),
                                                 stop=(kt == nkt - 1))
                            evict_fn(c0, c1, mt, ps, mw)

            # ================= projector =================
            with (
                tc.tile_pool(name="proj", bufs=1) as projp,
                tc.tile_pool(name="proj_ev", bufs=3) as projev,
            ):
                memT_sb = projp.tile([P, DMKT, MLEN], f16)
                nc.sync.dma_start(out=memT_sb[:], in_=memT_d[:])
                pb1_sb = projp.tile([P, PHKT], f32)
                pb2_sb = projp.tile([P, DKT], f32)
                nc.sync.dma_start(out=pb1_sb[:], in_=pb1_d[:])
                nc.sync.dma_start(out=pb2_sb[:], in_=pb2_d[:])
                gT = projp.tile([P, PHKT, MLEN], f16)

                def ev_g(c0, c1, mt, ps, mw):
                    nc.scalar.activation(gT[:, mt, c0:c1], ps[:], AF.Gelu,
                                         bias=pb1_sb[:, mt:mt + 1])
                stream_proj("pj1", pw1_d, PHKT, DMKT,
                            lambda kt, c0, c1: memT_sb[:, kt, c0:c1], ev_g)

                def ev_m(c0, c1, mt, ps, mw):
                    t = projev.tile([P, 512], f16, tag="mev")
                    nc.scalar.activation(t[:], ps[:], AF.Identity,
                                         bias=pb2_sb[:, mt:mt + 1])
                    nc.sync.dma_start(out=mem_par[:, mt, c0:c1], in_=t[:])
                stream_proj("pj2", pw2_d, DKT, PHKT,
                            lambda kt, c0, c1: gT[:, kt, c0:c1], ev_m)

                nc.gpsimd.collective_compute(
                    "AllReduce", ALU.add, ins=[mem_par[:]], outs=[mem_red[:]],
                    replica_groups=rg)

            # ================= block0: self-attention =================
            with (
                tc.tile_pool(name="b0norm", bufs=1) as b0_np,
                tc.tile_pool(name="b0act", bufs=1) as b0_act,
            ):
                rbc0, rbcq0, rT0 = emit_norm("n0", b0_np, 0, S,
                                             want_q=True, want_t=True)
                qkT0 = b0_act.tile([P, 2 * HSH, S], f16)
                v0_sb = b0_act.tile([P, SKT, DSH], f16)
                attn_o0 = b0_act.tile([P, HSH, S], f16)

                def ev_qk0(c0, c1, mt, ps, mw):
                    nc.scalar.activation(qkT0[:, mt, c0:c1], ps[:], AF.Copy)
                stream_proj("b0qk", wqk_d, 2 * HSH, DKT,
                            lambda kt, c0, c1: hT[:, kt, c0:c1], ev_qk0)

                # v projection: lhsT = hT seq slices, rhs = wv tiles
                with (
                    tc.tile_pool(name="b0vw", bufs=3) as vwp,
                    tc.tile_pool(name="b0vp", bufs=1, space="PSUM") as vps,
                ):
                    for half in range(2):
                        pss = [vps.tile([P, DSH], f32, tag=f"psv{i}",
                                        name=f"psv0_{half}_{i}")
                               for i in range(4)]
                        for kt in range(DKT):
                            wvt = vwp.tile([P, DSH], f16, tag="wvt")
                            nc.sync.dma_start(out=wvt[:], in_=wv_d[:, kt, :])
                            for i in range(4):
                                mt = half * 4 + i
                                nc.tensor.matmul(
                                    pss[i][:], hT[:, kt, mt * P:(mt + 1) * P],
                                    wvt[:], start=(kt == 0),
                                    stop=(kt == DKT - 1))
                        for i in range(4):
                            mt = half * 4 + i
                            nc.scalar.activation(v0_sb[:, mt, :], pss[i][:],
                                                 AF.Copy,
                                                 scale=rT0[:, mt:mt + 1])

                # rope on q and k tiles + norm/softmax scaling
                with (
                    tc.tile_pool(name="b0r", bufs=2) as rp,
                    tc.tile_pool(name="b0rp", bufs=2, space="PSUM") as rps,
                ):
                    for t in range(2 * HSH):
                        sc = rbcq0 if t < HSH else rbc0
                        for c0, c1 in CH:
                            psr = rps.tile([P, 512], f32, tag="psr")
                            nc.tensor.matmul(psr[:], rotM[:], qkT0[:, t, c0:c1],
                                             start=True, stop=True)
                            t2 = rp.tile([P, 512], f16, tag="t2")
                            nc.vector.tensor_mul(t2[:], psr[:], sinT[:, c0:c1])
                            t3 = rp.tile([P, 512], f16, tag="t3")
                            nc.vector.tensor_mul(t3[:], qkT0[:, t, c0:c1],
                                                 cosT[:, c0:c1])
                            nc.vector.tensor_add(t2[:], t2[:], t3[:])
                            nc.vector.tensor_mul(qkT0[:, t, c0:c1], t2[:],
                                                 sc[:, c0:c1])

                emit_attention("a0", qkT0, v0_sb, attn_o0)

                # o-projection, chunk-major; AllReduce per half
                with (
                    tc.tile_pool(name="b0oevp", bufs=3) as b0oev,
                    tc.tile_pool(name="b0o_w", bufs=4) as wp0,
                    tc.tile_pool(name="b0o_p", bufs=2, space="PSUM") as pp0,
                ):
                    if True:
                        for ci, (c0, c1) in enumerate(CH):
                            for mt in range(DKT):
                                wt = wp0.tile([P, HSH, P], f16, tag="wt")
                                nc.sync.dma_start(
                                    out=wt[:],
                                    in_=wo_d[:, :, mt * P:(mt + 1) * P])
                                ps = pp0.tile([P, 512], f32, tag="ps")
                                for kt in range(HSH):
                                    nc.tensor.matmul(ps[:], wt[:, kt, :],
                                                     attn_o0[:, kt, c0:c1],
                                                     start=(kt == 0),
                                                     stop=(kt == HSH - 1))
                                t_ = b0oev.tile([P, 512], f16, tag="oev")
                                nc.vector.scalar_tensor_tensor(
                                    t_[:], hT[:, mt, c0:c1], 1.0 / NCORES,
                                    ps[:], ALU.mult, ALU.add)
                                nc.sync.dma_start(out=blk_par[0][ci][:, mt, :],
                                                  in_=t_[:])
                            nc.gpsimd.collective_compute(
                                "AllReduce", ALU.add, ins=[blk_par[0][ci][:]],
                                outs=[blk_red[0][ci][:]], replica_groups=rg)

            # ===== block1 k/v projections (from SBUF-resident memory) =====
            # these run during block0's residual AllReduce
            with tc.tile_pool(name="b1act", bufs=1) as b1_act:
                qkT1 = b1_act.tile([P, 2 * HSH, S], f16)
                v1_sb = b1_act.tile([P, SKT, DSH], f16)
                attn_o1 = b1_act.tile([P, HSH, S], f16)

                with tc.tile_pool(name="memr", bufs=1) as memrp:
                    memR = memrp.tile([P, DKT, MLEN], f16)
                    nc.sync.dma_start(out=memR[:], in_=mem_red[:])

                    def ev_k1(c0, c1, mt, ps, mw):
                        nc.scalar.activation(qkT1[:, HSH + mt, c0:c1], ps[:],
                                             AF.Copy)
                    stream_proj("b1k", cwqk_d.ap()[:, :, DSH:2 * DSH], HSH, DKT,
                                lambda kt, c0, c1: memR[:, kt, c0:c1], ev_k1)

                    with (
                        tc.tile_pool(name="b1vw", bufs=3) as vwp1,
                        tc.tile_pool(name="b1vp", bufs=1, space="PSUM") as vps1,
                    ):
                        for half in range(2):
                            pss = [vps1.tile([P, DSH], f32, tag=f"psv{i}",
                                             name=f"psv1_{half}_{i}")
                                   for i in range(4)]
                            for kt in range(DKT):
                                wvt = vwp1.tile([P, DSH], f16, tag="wvt")
                                nc.sync.dma_start(out=wvt[:], in_=cwv_d[:, kt, :])
                                for i in range(4):
                                    mt = half * 4 + i
                                    nc.tensor.matmul(
                                        pss[i][:],
                                        memR[:, kt, mt * P:(mt + 1) * P],
                                        wvt[:], start=(kt == 0),
                                        stop=(kt == DKT - 1))
                            for i in range(4):
                                mt = half * 4 + i
                                nc.scalar.activation(v1_sb[:, mt, :], pss[i][:],
                                                     AF.Copy)

                # ===== reload hT halves; norm1 + q1, per chunk so chunk-a
                # work never queues behind chunk-b dependencies =====
                with tc.tile_pool(name="b1norm", bufs=1) as b1_np:
                    for ci, (c0, c1) in enumerate(CH):
                        nc.sync.dma_start(out=hT[:, :, c0:c1],
                                          in_=blk_red[0][ci][:])
                        _, rbcq1, _ = emit_norm(f"n1{ci}", b1_np, c0, c1,
                                                want_q=True)

                        def ev_q1(cc0, cc1, mt, ps, mw, rb=rbcq1):
                            nc.vector.tensor_mul(qkT1[:, mt, cc0:cc1], ps[:],
                                                 rb[:])
                        stream_proj(f"b1q{ci}", cwqk_d.ap()[:, :, 0:DSH],
                                    HSH, DKT,
                                    lambda kt, cc0, cc1: hT[:, kt, cc0:cc1],
                                    ev_q1, chunks=(CH[ci],))

                    emit_attention("a1", qkT1, v1_sb, attn_o1)

                    # o-projection, chunk-major; AllReduce per half
                    with (
                        tc.tile_pool(name="b1oev", bufs=3) as b1oev,
                        tc.tile_pool(name="b1o_w", bufs=4) as wp1,
                        tc.tile_pool(name="b1o_p", bufs=2, space="PSUM") as pp1,
                    ):
                        for ci, (c0, c1) in enumerate(CH):
                            for mt in range(DKT):
                                wt = wp1.tile([P, HSH, P], f16, tag="wt")
                                nc.sync.dma_start(
                                    out=wt[:],
                                    in_=cwo_d[:, :, mt * P:(mt + 1) * P])
                                ps = pp1.tile([P, 512], f32, tag="ps")
                                for kt in range(HSH):
                                    nc.tensor.matmul(ps[:], wt[:, kt, :],
                                                     attn_o1[:, kt, c0:c1],
                                                     start=(kt == 0),
                                                     stop=(kt == HSH - 1))
                                t_ = b1oev.tile([P, 512], f16, tag="oev")
                                nc.vector.scalar_tensor_tensor(
                                    t_[:], hT[:, mt, c0:c1], 1.0 / NCORES,
                                    ps[:], ALU.mult, ALU.add)
                                nc.sync.dma_start(out=blk_par[1][ci][:, mt, :],
                                                  in_=t_[:])
                            nc.gpsimd.collective_compute(
                                "AllReduce", ALU.add, ins=[blk_par[1][ci][:]],
                                outs=[blk_red[1][ci][:]], replica_groups=rg)

            # ================= MLP (swiglu fused into evictions) ============
            with (
                tc.tile_pool(name="mlpnorm", bufs=1) as mlp_np,
                tc.tile_pool(name="mlpact", bufs=1) as mlp_act,
                tc.tile_pool(name="mlptmp", bufs=2) as mlp_tmp,
            ):
                guT = mlp_act.tile([P, FFKT, S], f16)
                sg_t = {}
                for ci, (c0, c1) in enumerate(CH):
                    nc.sync.dma_start(out=hT[:, :, c0:c1], in_=blk_red[1][ci][:])
                    rbc2, _, _ = emit_norm(f"n2{ci}", mlp_np, c0, c1)

                    def ev_gu(cc0, cc1, mt, ps, mw, rb=rbc2):
                        ft = mt // 2
                        if mt % 2 == 0:     # gate tile: silu(g * rbc2)
                            gs = mlp_tmp.tile([P, 512], f16, tag="gs")
                            nc.vector.tensor_mul(gs[:], ps[:], rb[:])
                            sg = mlp_tmp.tile([P, 512], f16, tag="sg",
                                              name=f"sg_{cc0}_{ft}")
                            nc.scalar.activation(sg[:], gs[:], AF.Silu)
                            sg_t[(cc0, ft)] = sg
                        else:               # up tile: (u * rbc2) * silu_gate
                            us = mlp_tmp.tile([P, 512], f16, tag="us")
                            nc.vector.tensor_mul(us[:], ps[:], rb[:])
                            nc.vector.tensor_mul(guT[:, ft, cc0:cc1],
                                                 sg_t.pop((cc0, ft))[:], us[:])
                    stream_proj(f"mgu{ci}", wgu_d, 2 * FFKT, DKT,
                                lambda kt, cc0, cc1: hT[:, kt, cc0:cc1],
                                ev_gu, chunks=(CH[ci],))

                # down projection, chunk-major; AllReduce per half
                with (
                    tc.tile_pool(name="mdev", bufs=3) as mdev,
                    tc.tile_pool(name="md_w", bufs=3) as wpd,
                    tc.tile_pool(name="md_p", bufs=2, space="PSUM") as ppd,
                ):
                    for ci, (c0, c1) in enumerate(CH):
                        for mt in range(DKT):
                            wt = wpd.tile([P, FFKT, P], f16, tag="wt")
                            nc.sync.dma_start(
                                out=wt[:], in_=wd_d[:, :, mt * P:(mt + 1) * P])
                            ps = ppd.tile([P, 512], f32, tag="ps")
                            for kt in range(FFKT):
                                nc.tensor.matmul(ps[:], wt[:, kt, :],
                                                 guT[:, kt, c0:c1],
                                                 start=(kt == 0),
                                                 stop=(kt == FFKT - 1))
                            t_ = mdev.tile([P, 512], f16, tag="dev")
                            nc.vector.scalar_tensor_tensor(
                                t_[:], hT[:, mt, c0:c1], 1.0 / NCORES, ps[:],
                                ALU.mult, ALU.add)
                            nc.sync.dma_start(out=blk_par[2][ci][:, mt, :],
                                              in_=t_[:])
                        nc.gpsimd.collective_compute(
                            "AllReduce", ALU.add, ins=[blk_par[2][ci][:]],
                            outs=[blk_red[2][ci][:]], replica_groups=rg)

            # ================= lm head =================
            with (
                tc.tile_pool(name="lmhnorm", bufs=1) as lmh_np,
                tc.tile_pool(name="lmhev", bufs=3) as lev,
            ):
                for ci, (c0, c1) in enumerate(CH):
                    nc.sync.dma_start(out=hT[:, :, c0:c1], in_=blk_red[2][ci][:])
                    rbc3, _, _ = emit_norm(f"n3{ci}", lmh_np, c0, c1)

                    def ev_l(cc0, cc1, mt, ps, mw, rb=rbc3):
                        t_ = lev.tile([P, 512], f32, tag="lev")
                        nc.vector.tensor_mul(t_[0:mw, :], ps[0:mw, :],
                                             rb[0:mw, :])
                        nc.sync.dma_start(
                            out=logits_d.ap()[mt * P:mt * P + mw, cc0:cc1],
                            in_=t_[0:mw, :])
                    stream_proj(f"lh{ci}", lmh_d, (VSH + P - 1) // P, DKT,
                                lambda kt, cc0, cc1: hT[:, kt, cc0:cc1], ev_l,
                                total=VSH, chunks=(CH[ci],))

    nc.compile()
    return nc


def _part(x, kt):
    """[R, C] -> [128, R//128, C] with row = kt_idx*128 + p."""
    R, C = x.shape
    return np.ascontiguousarray(x.reshape(kt, P, C).transpose(1, 0, 2))


def kernel(**inputs):
    inp = {k: np.asarray(v) for k, v in inputs.items()}
    ids = inp["input_ids"].astype(np.int64)[0]          # [S]
    memory = inp["memory"].astype(np.float32)[0]        # [MLEN, DM]
    f = np.float32

    ln1 = inp["ln1"].astype(f)
    lnc = inp["lnc"].astype(f)
    ln2 = inp["ln2"].astype(f)
    lnf = inp["lnf"].astype(f)

    h0 = inp["embed"].astype(f)[ids]                    # [S, D]
    hT0 = _part(h0.T.astype(np.float16), DKT)           # [128, 32, S]
    memT = _part(memory.T.astype(np.float16), DMKT)     # [128, 8, MLEN]

    # RoPE tables (transposed layout [DH, S])
    inv = 1.0 / (10000.0 ** (np.arange(0, DH, 2, dtype=f) / DH))
    t = np.arange(S, dtype=f)
    freqs = np.outer(t, inv)                            # [S, DH//2]
    emb = np.concatenate([freqs, freqs], axis=1)        # [S, DH]
    cosT = np.cos(emb).T.astype(np.float16)             # [DH, S]
    sinT = np.sin(emb).T.astype(np.float16)
    rotM = np.zeros((P, P), dtype=np.float16)           # rotM[k,d]: rot_half
    rotM[np.arange(64) + 64, np.arange(64)] = -1.0      # out[d<64] = -in[d+64]
    rotM[np.arange(64), np.arange(64) + 64] = 1.0       # out[d>=64] = in[d-64]
    maskT = np.triu(np.ones((P, P), dtype=np.float16))  # [key p, query col]

    wq = inp["wq"].astype(f) * ln1[:, None]
    wk = inp["wk"].astype(f) * ln1[:, None]
    wv = inp["wv"].astype(f) * ln1[:, None]
    cwq = inp["cwq"].astype(f) * lnc[:, None]
    cwk = inp["cwk"].astype(f)
    cwv = inp["cwv"].astype(f)
    wg = inp["wg"].astype(f) * ln2[:, None]
    wu = inp["wu"].astype(f) * ln2[:, None]
    lmh = inp["lm_head"].astype(f) * lnf[:, None]
    wo = inp["wo"].astype(f)
    cwo = inp["cwo"].astype(f)
    wd = inp["wd"].astype(f)
    pw1 = inp["p_w1"].astype(f)
    pw2 = inp["p_w2"].astype(f)
    pb1 = inp["p_b1"].astype(f)
    pb2 = inp["p_b2"].astype(f)

    h16 = np.float16
    in_maps = []
    for c in range(NCORES):
        ds = slice(c * DSH, (c + 1) * DSH)
        ffs = slice(c * FFSH, (c + 1) * FFSH)
        phs = slice(c * PHS, (c + 1) * PHS)
        vs = slice(c * VSH, (c + 1) * VSH)

        # gate/up column tiles interleaved: g0,u0,g1,u1,...
        g_pad = np.zeros((D, FFPAD), dtype=h16)
        g_pad[:, 0:FFSH] = wg[:, ffs].astype(h16)
        u_pad = np.zeros((D, FFPAD), dtype=h16)
        u_pad[:, 0:FFSH] = wu[:, ffs].astype(h16)
        wgu_c = np.empty((D, 2 * FFPAD), dtype=h16)
        for ft in range(FFKT):
            wgu_c[:, (2 * ft) * P:(2 * ft + 1) * P] = \
                g_pad[:, ft * P:(ft + 1) * P]
            wgu_c[:, (2 * ft + 1) * P:(2 * ft + 2) * P] = \
                u_pad[:, ft * P:(ft + 1) * P]
        wd_c = np.zeros((FFPAD, D), dtype=h16)
        wd_c[0:FFSH] = wd[ffs, :].astype(h16)

        m = {
            "hT0": hT0, "memT": memT,
            "pw1": _part(pw1[:, phs].astype(h16), DMKT),
            "pw2": _part(pw2[phs, :].astype(h16), PHKT),
            "pb1": np.ascontiguousarray(pb1[phs].reshape(PHKT, P).T.astype(f)),
            "pb2": np.ascontiguousarray((pb2 / NCORES).reshape(DKT, P).T.astype(f)),
            "wqk": _part(np.concatenate([wq[:, ds], wk[:, ds]], axis=1).astype(h16), DKT),
            "wv": _part(wv[:, ds].astype(h16), DKT),
            "wo": _part(wo[ds, :].astype(h16), DSH // P),
            "cwqk": _part(np.concatenate([cwq[:, ds], cwk[:, ds]], axis=1).astype(h16), DKT),
            "cwv": _part(cwv[:, ds].astype(h16), DKT),
            "cwo": _part(cwo[ds, :].astype(h16), DSH // P),
            "wgu": _part(wgu_c, DKT),
            "wd": _part(wd_c, FFKT),
            "lmh": _part(lmh[:, vs].astype(h16), DKT),
            "cosT": cosT, "sinT": sinT, "rotM": rotM, "maskT": maskT,
        }
        in_maps.append(m)

    if "nc" not in _prog_cache:
        _prog_cache["nc"] = _build_program()
    nc = _prog_cache["nc"]

    res = run_bass_kernel_spmd(nc, in_maps, list(range(NCORES)))
    global LAST_RESULTS
    LAST_RESULTS = res
    logits = np.concatenate([r["logitsT"].T for r in res.results], axis=1)
    return logits.reshape(B, S, V).astype(np.float32)


if __name__ == "__main__":
    # quick build check
    nc = _build_program()
    print("program built ok")


# revision 31
# speedup vs baseline: 1.1556x; 1.0214x over previous
# Trainium2 Bass kernel for nn_Decoder_51582557225714.
# 8-way tensor-parallel single-layer decoder with cross-attention.
#
# Sharding (per core c of 8):
#  - q/k/v/o, cross q/k/v/o: column-shard by head (4 heads = 512 cols per core),
#    o/cwo row-sharded; partial outputs AllReduced.
#  - MLP gate/up column-shard (1376 -> padded 1408 cols), down row-shard, AllReduce.
#  - projector: p_w1 column-shard (1024 cols of PH), p_w2 row-shard, AllReduce.
#  - lm_head vocab-shard (1000 cols per core), gathered on host.
#  - embedding gather + all input sharding/transposition done host-side.
#
# v2 schedule (keeps TensorE dense + overlaps every collective):
#  - projector runs first (its matmuls warm the PE while hT0 uploads);
#    mem AllReduce rides under block0's projections.
#  - projected memory is AllReduced once and then kept resident in SBUF, so
#    cross-attention k/v projections stream at full rate (no HBM re-reads).
#  - cross k/v projections are emitted right after block0's o-projection so
#    they execute during block0's residual AllReduce.
#  - every residual AllReduce is split into two S-halves; producers evict
#    chunk-major so the first half reduces while the second is computed, and
#    consumers (norm + next projection) start on half 0 while half 1 reduces.
#  - attention is causally trimmed (no matmuls/exp on the all-zero triangle)
#    and processed chunk-major with per-kt probability tiles.
#  - swiglu is fused into the gate/up evictions (wgu tiles interleaved g,u).
# All activations kept TRANSPOSED ([feature, seq]) on device; fp16 data with
# fp32 PSUM accumulation; rmsnorm folded into weights (ln scale) + column
# rescale (rsqrt); softmax without max-subtraction, exp(score-5).

import math
import numpy as np

import concourse.bass as bass
import concourse.mybir as mybir
import concourse.tile as tile
from concourse import bacc
from concourse.bass_utils import run_bass_kernel_spmd

P = 128
NCORES = 8
B, S, MLEN = 1, 1024, 1024
D, H, DH, FF = 4096, 32, 128, 11008
V, DM, PH = 8000, 1024, 8192
EPS = 1e-6

DKT = D // P            # 32 k-tiles over D
DMKT = DM // P          # 8
HSH = H // NCORES       # 4 heads per core
DSH = HSH * DH          # 512
FFSH = FF // NCORES     # 1376
FFPAD = 1408            # padded to 11*128
FFKT = FFPAD // P       # 11
PHS = PH // NCORES      # 1024
PHKT = PHS // P         # 8
VSH = V // NCORES       # 1000
SKT = S // P            # 8
CH = ((0, 512), (512, 1024))

f32 = mybir.dt.float32
f16 = mybir.dt.float16
AF = mybir.ActivationFunctionType
ALU = mybir.AluOpType

_prog_cache = {}
LAST_RESULTS = None  # BassKernelResults of the most recent run (for harness use)


def _build_program():
    nc = bacc.Bacc("TRN2", target_bir_lowering=False, debug=False,
                   enable_asserts=True, num_devices=NCORES)

    # ---- I/O declarations (per core) ----
    def din(name, shape, dt=f16):
        return nc.dram_tensor(name, shape, dt, kind="ExternalInput")

    hT0_d = din("hT0", [P, DKT, S])
    memT_d = din("memT", [P, DMKT, MLEN])
    pw1_d = din("pw1", [P, DMKT, PHS])
    pw2_d = din("pw2", [P, PHKT, D])
    pb1_d = din("pb1", [P, PHKT], f32)
    pb2_d = din("pb2", [P, DKT], f32)          # p_b2 / 8
    wqk_d = din("wqk", [P, DKT, 2 * DSH])
    wv_d = din("wv", [P, DKT, DSH])
    wo_d = din("wo", [P, DSH // P, D])
    cwqk_d = din("cwqk", [P, DKT, 2 * DSH])
    cwv_d = din("cwv", [P, DKT, DSH])
    cwo_d = din("cwo", [P, DSH // P, D])
    wgu_d = din("wgu", [P, DKT, 2 * FFPAD])    # tiles interleaved g0,u0,g1,u1,...
    wd_d = din("wd", [P, FFKT, D])
    lmh_d = din("lmh", [P, DKT, VSH])
    cosT_d = din("cosT", [P, S])
    sinT_d = din("sinT", [P, S])
    rotM_d = din("rotM", [P, P])
    maskT_d = din("maskT", [P, P])

    logits_d = nc.dram_tensor("logitsT", [VSH, S], f32, kind="ExternalOutput")

    # collective bounce buffers
    mem_par = nc.dram_tensor("mem_par", [P, DKT, MLEN], f16)
    mem_red = nc.dram_tensor("mem_red", [P, DKT, MLEN], f16, addr_space="Shared")
    blk_par = [[nc.dram_tensor(f"blk_par{i}_{c}", [P, DKT, 512], f16)
                for c in range(2)] for i in range(3)]
    blk_red = [[nc.dram_tensor(f"blk_red{i}_{c}", [P, DKT, 512], f16,
                               addr_space="Shared")
                for c in range(2)] for i in range(3)]
    scratch_rs = nc.dram_tensor("rs_scratch", [S], f32)

    rg = [list(range(NCORES))]

    with tile.TileContext(nc) as tc:
        with (
            tc.tile_pool(name="persist", bufs=1) as persist,
            # Global scratch pools, allocated once for the whole kernel.
            # Phase-local pools land in just-released SBUF zones and their
            # first weight DMA waits on every reader of the previous phase
            # (zone-overlap dep) — persistent pools give per-slot deps, so
            # weight prefetch crosses phase boundaries.
            tc.tile_pool(name="gw32", bufs=2) as gw32,
            tc.tile_pool(name="gw11", bufs=3) as gw11,
            tc.tile_pool(name="gw8", bufs=2) as gw8,
            tc.tile_pool(name="gw4", bufs=4) as gw4,
            tc.tile_pool(name="gev", bufs=3) as gev,
            tc.tile_pool(name="gsq", bufs=2) as gsq,
            tc.tile_pool(name="gat", bufs=4) as gat,
            tc.tile_pool(name="gat2", bufs=2) as gat2,
            tc.tile_pool(name="gwv", bufs=3) as gwv,
        ):
            WPOOL = {32: gw32, 11: gw11, 8: gw8, 4: gw4}
            hT = persist.tile([P, DKT, S], f16)
            for q in range(4):
                nc.sync.dma_start(out=hT[:, 8 * q:8 * q + 8, :],
                                  in_=hT0_d[:, 8 * q:8 * q + 8, :])
            cosT = persist.tile([P, S], f16)
            sinT = persist.tile([P, S], f16)
            rotM = persist.tile([P, P], f16)
            maskT = persist.tile([P, P], f16)
            ones = persist.tile([P, 1], f16)
            nc.sync.dma_start(out=cosT[:], in_=cosT_d[:])
            nc.sync.dma_start(out=sinT[:], in_=sinT_d[:])
            nc.sync.dma_start(out=rotM[:], in_=rotM_d[:])
            nc.sync.dma_start(out=maskT[:], in_=maskT_d[:])
            nc.vector.memset(ones[:], 1.0)
            eps_t = persist.tile([1, 1], f32)
            nc.vector.memset(eps_t[:], EPS)
            nexp_t = persist.tile([P, 1], f32)
            nc.vector.memset(nexp_t[:], -5.0)

            # ---------- shared emitters ----------
            def emit_norm(nm, pool, c0, c1, want_q=False, want_t=False):
                """rsqrt(mean(h^2)+eps) over cols [c0,c1). Returns
                (rbc [P,w] f32, rbcq or None, rT [P,SKT] f32 or None)."""
                w = c1 - c0
                with tc.tile_pool(name=f"{nm}_sp", bufs=1,
                                  space="PSUM") as sps:
                    ps = sps.tile([1, w], f32)
                    for kt in range(DKT):
                        hsq = gsq.tile([P, w], f16, tag="hsq",
                                       name=f"hsq_{nm}_{kt}")
                        src = hT[:, kt, c0:c1]
                        # split the squares across ACT and DVE
                        if kt % 2 == 0:
                            nc.scalar.activation(hsq[:], src, AF.Square)
                        else:
                            nc.vector.tensor_mul(hsq[:], src, src)
                        # one psum bank (512 f32) per matmul
                        for s0 in range(0, w, 512):
                            s1 = min(w, s0 + 512)
                            nc.tensor.matmul(ps[0:1, s0:s1], ones[:, 0:1],
                                             hsq[:, s0:s1],
                                             start=(kt == 0),
                                             stop=(kt == DKT - 1))
                    row = gsq.tile([1, w], f32, tag="row", name=f"row_{nm}")
                    nc.scalar.activation(row[:], ps[0:1, :], AF.Sqrt,
                                         scale=1.0 / D, bias=eps_t[0:1, 0:1])
                    rrow = gsq.tile([1, w], f32, tag="rrow", name=f"rrow_{nm}")
                    nc.vector.reciprocal(rrow[:], row[:])

                    rbc = pool.tile([P, w], f32, tag=f"{nm}_rbc")
                    nc.gpsimd.partition_broadcast(rbc[:], rrow[0:1, :])
                    rbcq = None
                    if want_q:
                        rbcq = pool.tile([P, w], f32, tag=f"{nm}_rbcq")
                        nc.vector.tensor_scalar_mul(rbcq[:], rbc[:],
                                                    1.0 / math.sqrt(DH))
                    rT = None
                    if want_t:
                        nc.sync.dma_start(out=scratch_rs[:], in_=rrow[0:1, :])
                        rT = pool.tile([P, SKT], f32, tag=f"{nm}_rT")
                        nc.sync.dma_start(
                            out=rT[:],
                            in_=scratch_rs.ap().rearrange("(kt p) -> p kt", p=P))
                return rbc, rbcq, rT

            def emit_attention(nm, qkT, v_sb, attn_o):
                """Causally-trimmed attention, chunk-major over q columns.
                qkT [P, 2*HSH, S] (q tiles 0..HSH-1 scaled, k tiles HSH..).
                v_sb [P, SKT, DSH]. attn_o [P, HSH, S] f16."""
                with (
                    tc.tile_pool(name=f"{nm}_ps", bufs=2, space="PSUM") as psp,
                    tc.tile_pool(name=f"{nm}_po", bufs=2, space="PSUM") as pop,
                    tc.tile_pool(name=f"{nm}_pc", bufs=2, space="PSUM") as pcp,
                ):
                    for ci, (c0, c1) in enumerate(CH):
                        kt_last = c1 // P - 1
                        for h in range(HSH):
                            ps_o = pop.tile([P, 512], f32, tag="ps_o")
                            ps_cs = pcp.tile([1, 512], f32, tag="ps_cs")
                            for kt in range(SKT):
                                n0 = kt * P
                                a0 = max(c0, n0)
                                if a0 >= c1:
                                    continue
                                w = c1 - a0
                                ps_s = psp.tile([P, 512], f32, tag="ps_s")
                                nc.tensor.matmul(
                                    ps_s[:, 0:w], qkT[:, HSH + h, n0:n0 + P],
                                    qkT[:, h, a0:c1], start=True, stop=True)
                                pt = gat.tile([P, w], f16, tag="pt",
                                              name=f"pt_{nm}_{ci}_{kt}_{h}")
                                nc.scalar.activation(pt[:], ps_s[:, 0:w], AF.Exp,
                                                     bias=nexp_t[:, 0:1])
                                if a0 == n0:
                                    nc.vector.tensor_mul(pt[:, 0:P], pt[:, 0:P],
                                                         maskT[:])
                                st = (kt == 0)
                                sp = (kt == kt_last)
                                nc.tensor.matmul(ps_cs[0:1, a0 - c0:c1 - c0],
                                                 ones[:, 0:1], pt[:],
                                                 start=st, stop=sp)
                                nc.tensor.matmul(
                                    ps_o[:, a0 - c0:c1 - c0],
                                    v_sb[:, kt, h * DH:(h + 1) * DH], pt[:],
                                    start=st, stop=sp)
                            rrow = gat2.tile([1, 512], f32, tag="rrow",
                                             name=f"arow_{nm}_{ci}_{h}")
                            nc.vector.reciprocal(rrow[:], ps_cs[0:1, :])
                            rbc = gat2.tile([P, 512], f32, tag="rbc",
                                            name=f"abc_{nm}_{ci}_{h}")
                            nc.gpsimd.partition_broadcast(rbc[:], rrow[0:1, :])
                            nc.vector.tensor_mul(attn_o[:, h, c0:c1], ps_o[:],
                                                 rbc[:])

            def stream_proj(nm, w_dram, nmt, nkt, rhs_fn, evict_fn,
                            chunks=CH, wbufs=2, total=None):
                """Chunk-major weight-streaming projection.
                out[mt, c0:c1] = sum_kt w[:, kt, mslice].T @ rhs(kt, c0, c1).
                evict_fn(c0, c1, mt, ps, mw) consumes psum [mw, c1-c0].
                Weight tiles come from the persistent per-nkt pools."""
                if total is None:
                    total = nmt * P
                wp = WPOOL[nkt]
                with tc.tile_pool(name=f"{nm}_p", bufs=2, space="PSUM") as pp:
                    for c0, c1 in chunks:
                        for mt in range(nmt):
                            m0 = mt * P
                            mw = min(P, total - m0)
                            wt = wp.tile([P, nkt, P], f16, tag=f"wt{nkt}",
                                         name=f"wt_{nm}_{c0}_{mt}")
                            nc.sync.dma_start(out=wt[:, :, 0:mw],
                                              in_=w_dram[:, :, m0:m0 + mw])
                            ps = pp.tile([P, 512], f32, tag="ps")
                            for kt in range(nkt):
                                nc.tensor.matmul(ps[0:mw, 0:c1 - c0],
                                                 wt[:, kt, 0:mw],
                                                 rhs_fn(kt, c0, c1),
                                                 start=(kt == 0),
                                                 stop=(kt == nkt - 1))
                            evict_fn(c0, c1, mt, ps, mw)

            # ================= projector =================
            with tc.tile_pool(name="proj", bufs=1) as projp:
                memT_sb = projp.tile([P, DMKT, MLEN], f16)
                nc.sync.dma_start(out=memT_sb[:], in_=memT_d[:])
                pb1_sb = projp.tile([P, PHKT], f32)
                pb2_sb = projp.tile([P, DKT], f32)
                nc.sync.dma_start(out=pb1_sb[:], in_=pb1_d[:])
                nc.sync.dma_start(out=pb2_sb[:], in_=pb2_d[:])
                gT = projp.tile([P, PHKT, MLEN], f16)

                def ev_g(c0, c1, mt, ps, mw):
                    nc.scalar.activation(gT[:, mt, c0:c1], ps[:], AF.Gelu,
                                         bias=pb1_sb[:, mt:mt + 1])
                stream_proj("pj1", pw1_d, PHKT, DMKT,
                            lambda kt, c0, c1: memT_sb[:, kt, c0:c1], ev_g)

                def ev_m(c0, c1, mt, ps, mw):
                    t = gev.tile([P, 512], f16, tag="ev16",
                                 name=f"mev_{c0}_{mt}")
                    nc.scalar.activation(t[:], ps[:], AF.Identity,
                                         bias=pb2_sb[:, mt:mt + 1])
                    nc.sync.dma_start(out=mem_par[:, mt, c0:c1], in_=t[:])
                stream_proj("pj2", pw2_d, DKT, PHKT,
                            lambda kt, c0, c1: gT[:, kt, c0:c1], ev_m)

                nc.gpsimd.collective_compute(
                    "AllReduce", ALU.add, ins=[mem_par[:]], outs=[mem_red[:]],
                    replica_groups=rg)

            # ================= block0: self-attention =================
            with (
                tc.tile_pool(name="b0norm", bufs=1) as b0_np,
                tc.tile_pool(name="b0act", bufs=1) as b0_act,
            ):
                rbc0, rbcq0, rT0 = emit_norm("n0", b0_np, 0, S,
                                             want_q=True, want_t=True)
                qkT0 = b0_act.tile([P, 2 * HSH, S], f16)
                v0_sb = b0_act.tile([P, SKT, DSH], f16)

                def ev_qk0(c0, c1, mt, ps, mw):
                    nc.scalar.activation(qkT0[:, mt, c0:c1], ps[:], AF.Copy)
                stream_proj("b0qk", wqk_d, 2 * HSH, DKT,
                            lambda kt, c0, c1: hT[:, kt, c0:c1], ev_qk0)

                # v projection: lhsT = hT seq slices, rhs = wv tiles
                with tc.tile_pool(name="b0vp", bufs=1, space="PSUM") as vps:
                    for half in range(2):
                        pss = [vps.tile([P, DSH], f32, tag=f"psv{i}",
                                        name=f"psv0_{half}_{i}")
                               for i in range(4)]
                        for kt in range(DKT):
                            wvt = gwv.tile([P, DSH], f16, tag="wvt",
                                           name=f"wvt0_{half}_{kt}")
                            nc.sync.dma_start(out=wvt[:], in_=wv_d[:, kt, :])
                            for i in range(4):
                                mt = half * 4 + i
                                nc.tensor.matmul(
                                    pss[i][:], hT[:, kt, mt * P:(mt + 1) * P],
                                    wvt[:], start=(kt == 0),
                                    stop=(kt == DKT - 1))
                        for i in range(4):
                            mt = half * 4 + i
                            nc.scalar.activation(v0_sb[:, mt, :], pss[i][:],
                                                 AF.Copy,
                                                 scale=rT0[:, mt:mt + 1])

                # rope on q and k tiles + norm/softmax scaling
                with (
                    tc.tile_pool(name="b0r", bufs=2) as rp,
                    tc.tile_pool(name="b0rp", bufs=2, space="PSUM") as rps,
                ):
                    for t in range(2 * HSH):
                        sc = rbcq0 if t < HSH else rbc0
                        for c0, c1 in CH:
                            psr = rps.tile([P, 512], f32, tag="psr")
                            nc.tensor.matmul(psr[:], rotM[:], qkT0[:, t, c0:c1],
                                             start=True, stop=True)
                            t2 = rp.tile([P, 512], f16, tag="t2")
                            nc.vector.tensor_mul(t2[:], psr[:], sinT[:, c0:c1])
                            t3 = rp.tile([P, 512], f16, tag="t3")
                            nc.vector.tensor_mul(t3[:], qkT0[:, t, c0:c1],
                                                 cosT[:, c0:c1])
                            nc.vector.tensor_add(t2[:], t2[:], t3[:])
                            nc.vector.tensor_mul(qkT0[:, t, c0:c1], t2[:],
                                                 sc[:, c0:c1])

                # attention outputs overwrite the q slots of qkT0 (each
                # write touches only columns whose scores are already done)
                emit_attention("a0", qkT0, v0_sb, qkT0)

                # o-projection, chunk-major; AllReduce per half
                with tc.tile_pool(name="b0o_p", bufs=2, space="PSUM") as pp0:
                    for ci, (c0, c1) in enumerate(CH):
                        for mt in range(DKT):
                            wt = gw4.tile([P, HSH, P], f16, tag="wt4",
                                          name=f"wo0_{ci}_{mt}")
                            nc.sync.dma_start(
                                out=wt[:],
                                in_=wo_d[:, :, mt * P:(mt + 1) * P])
                            ps = pp0.tile([P, 512], f32, tag="ps")
                            for kt in range(HSH):
                                nc.tensor.matmul(ps[:], wt[:, kt, :],
                                                 qkT0[:, kt, c0:c1],
                                                 start=(kt == 0),
                                                 stop=(kt == HSH - 1))
                            t_ = gev.tile([P, 512], f16, tag="ev16",
                                          name=f"oev0_{ci}_{mt}")
                            nc.vector.scalar_tensor_tensor(
                                t_[:], hT[:, mt, c0:c1], 1.0 / NCORES,
                                ps[:], ALU.mult, ALU.add)
                            nc.sync.dma_start(out=blk_par[0][ci][:, mt, :],
                                              in_=t_[:])
                        nc.gpsimd.collective_compute(
                            "AllReduce", ALU.add, ins=[blk_par[0][ci][:]],
                            outs=[blk_red[0][ci][:]], replica_groups=rg)

            # ===== block1 k/v projections (from SBUF-resident memory) =====
            # these run during block0's residual AllReduce
            with tc.tile_pool(name="b1act", bufs=1) as b1_act:
                qkT1 = b1_act.tile([P, 2 * HSH, S], f16)
                v1_sb = b1_act.tile([P, SKT, DSH], f16)

                # memory processed in two column halves to halve SBUF
                # residency; each half feeds both the k columns and the
                # v seq-tiles that live in those columns.
                with (
                    tc.tile_pool(name="memr", bufs=1) as memrp,
                    tc.tile_pool(name="b1vp", bufs=1, space="PSUM") as vps1,
                ):
                    for ci, (c0, c1) in enumerate(CH):
                        memR = memrp.tile([P, DKT, 512], f16, tag="memR",
                                          name=f"memR{ci}")
                        nc.sync.dma_start(out=memR[:],
                                          in_=mem_red.ap()[:, :, c0:c1])

                        def ev_k1(cc0, cc1, mt, ps, mw):
                            nc.scalar.activation(qkT1[:, HSH + mt, cc0:cc1],
                                                 ps[:], AF.Copy)
                        stream_proj(f"b1k{ci}",
                                    cwqk_d.ap()[:, :, DSH:2 * DSH], HSH, DKT,
                                    lambda kt, cc0, cc1:
                                        memR[:, kt, 0:cc1 - cc0],
                                    ev_k1, chunks=(CH[ci],))

                        pss = [vps1.tile([P, DSH], f32, tag=f"psv{i}",
                                         name=f"psv1_{ci}_{i}")
                               for i in range(4)]
                        for kt in range(DKT):
                            wvt = gwv.tile([P, DSH], f16, tag="wvt",
                                           name=f"wvt1_{ci}_{kt}")
                            nc.sync.dma_start(out=wvt[:], in_=cwv_d[:, kt, :])
                            for i in range(4):
                                nc.tensor.matmul(
                                    pss[i][:],
                                    memR[:, kt, i * P:(i + 1) * P],
                                    wvt[:], start=(kt == 0),
                                    stop=(kt == DKT - 1))
                        for i in range(4):
                            mt = 4 * ci + i
                            nc.scalar.activation(v1_sb[:, mt, :], pss[i][:],
                                                 AF.Copy)

                # ===== reload hT halves; norm1 + q1, per chunk so chunk-a
                # work never queues behind chunk-b dependencies =====
                with tc.tile_pool(name="b1norm", bufs=1) as b1_np:
                    for ci, (c0, c1) in enumerate(CH):
                        nc.sync.dma_start(out=hT[:, :, c0:c1],
                                          in_=blk_red[0][ci][:])
                        _, rbcq1, _ = emit_norm(f"n1{ci}", b1_np, c0, c1,
                                                want_q=True)

                        def ev_q1(cc0, cc1, mt, ps, mw, rb=rbcq1):
                            nc.vector.tensor_mul(qkT1[:, mt, cc0:cc1], ps[:],
                                                 rb[:])
                        stream_proj(f"b1q{ci}", cwqk_d.ap()[:, :, 0:DSH],
                                    HSH, DKT,
                                    lambda kt, cc0, cc1: hT[:, kt, cc0:cc1],
                                    ev_q1, chunks=(CH[ci],))

                    emit_attention("a1", qkT1, v1_sb, qkT1)

                    # o-projection, chunk-major; AllReduce per half
                    with tc.tile_pool(name="b1o_p", bufs=2,
                                      space="PSUM") as pp1:
                        for ci, (c0, c1) in enumerate(CH):
                            for mt in range(DKT):
                                wt = gw4.tile([P, HSH, P], f16, tag="wt4",
                                              name=f"wo1_{ci}_{mt}")
                                nc.sync.dma_start(
                                    out=wt[:],
                                    in_=cwo_d[:, :, mt * P:(mt + 1) * P])
                                ps = pp1.tile([P, 512], f32, tag="ps")
                                for kt in range(HSH):
                                    nc.tensor.matmul(ps[:], wt[:, kt, :],
                                                     qkT1[:, kt, c0:c1],
                                                     start=(kt == 0),
                                                     stop=(kt == HSH - 1))
                                t_ = gev.tile([P, 512], f16, tag="ev16",
                                              name=f"oev1_{ci}_{mt}")
                                nc.vector.scalar_tensor_tensor(
                                    t_[:], hT[:, mt, c0:c1], 1.0 / NCORES,
                                    ps[:], ALU.mult, ALU.add)
                                nc.sync.dma_start(out=blk_par[1][ci][:, mt, :],
                                                  in_=t_[:])
                            nc.gpsimd.collective_compute(
                                "AllReduce", ALU.add, ins=[blk_par[1][ci][:]],
                                outs=[blk_red[1][ci][:]], replica_groups=rg)

            # ================= MLP (swiglu fused into evictions) ============
            with (
                tc.tile_pool(name="mlpnorm", bufs=1) as mlp_np,
                tc.tile_pool(name="mlpact", bufs=1) as mlp_act,
            ):
                guT = mlp_act.tile([P, FFKT, S], f16)
                sg_t = {}
                for ci, (c0, c1) in enumerate(CH):
                    nc.sync.dma_start(out=hT[:, :, c0:c1], in_=blk_red[1][ci][:])
                    rbc2, _, _ = emit_norm(f"n2{ci}", mlp_np, c0, c1)

                    def ev_gu(cc0, cc1, mt, ps, mw, rb=rbc2):
                        ft = mt // 2
                        if mt % 2 == 0:     # gate tile: silu(g * rbc2)
                            gs = gev.tile([P, 512], f16, tag="ev16",
                                          name=f"gs_{cc0}_{ft}")
                            nc.vector.tensor_mul(gs[:], ps[:], rb[:])
                            sg = gev.tile([P, 512], f16, tag="sg",
                                          name=f"sg_{cc0}_{ft}")
                            nc.scalar.activation(sg[:], gs[:], AF.Silu)
                            sg_t[(cc0, ft)] = sg
                        else:               # up tile: (u * rbc2) * silu_gate
                            us = gev.tile([P, 512], f16, tag="ev16",
                                          name=f"us_{cc0}_{ft}")
                            nc.vector.tensor_mul(us[:], ps[:], rb[:])
                            nc.vector.tensor_mul(guT[:, ft, cc0:cc1],
                                                 sg_t.pop((cc0, ft))[:], us[:])
                    stream_proj(f"mgu{ci}", wgu_d, 2 * FFKT, DKT,
                                lambda kt, cc0, cc1: hT[:, kt, cc0:cc1],
                                ev_gu, chunks=(CH[ci],))

                # down projection, chunk-major; AllReduce per half
                with tc.tile_pool(name="md_p", bufs=2, space="PSUM") as ppd:
                    for ci, (c0, c1) in enumerate(CH):
                        for mt in range(DKT):
                            wt = gw11.tile([P, FFKT, P], f16, tag="wt11",
                                           name=f"wd_{ci}_{mt}")
                            nc.sync.dma_start(
                                out=wt[:], in_=wd_d[:, :, mt * P:(mt + 1) * P])
                            ps = ppd.tile([P, 512], f32, tag="ps")
                            for kt in range(FFKT):
                                nc.tensor.matmul(ps[:], wt[:, kt, :],
                                                 guT[:, kt, c0:c1],
                                                 start=(kt == 0),
                                                 stop=(kt == FFKT - 1))
                            t_ = gev.tile([P, 512], f16, tag="ev16",
                                          name=f"dev_{ci}_{mt}")
                            nc.vector.scalar_tensor_tensor(
                                t_[:], hT[:, mt, c0:c1], 1.0 / NCORES, ps[:],
                                ALU.mult, ALU.add)
                            nc.sync.dma_start(out=blk_par[2][ci][:, mt, :],
                                              in_=t_[:])
                        nc.gpsimd.collective_compute(
                            "AllReduce", ALU.add, ins=[blk_par[2][ci][:]],
                            outs=[blk_red[2][ci][:]], replica_groups=rg)

            # ================= lm head =================
            with tc.tile_pool(name="lmhnorm", bufs=1) as lmh_np:
                for ci, (c0, c1) in enumerate(CH):
                    nc.sync.dma_start(out=hT[:, :, c0:c1], in_=blk_red[2][ci][:])
                    rbc3, _, _ = emit_norm(f"n3{ci}", lmh_np, c0, c1)

                    def ev_l(cc0, cc1, mt, ps, mw, rb=rbc3):
                        t_ = gev.tile([P, 512], f32, tag="lev",
                                      name=f"lev_{cc0}_{mt}")
                        nc.vector.tensor_mul(t_[0:mw, :], ps[0:mw, :],
                                             rb[0:mw, :])
                        nc.sync.dma_start(
                            out=logits_d.ap()[mt * P:mt * P + mw, cc0:cc1],
                            in_=t_[0:mw, :])
                    stream_proj(f"lh{ci}", lmh_d, (VSH + P - 1) // P, DKT,
                                lambda kt, cc0, cc1: hT[:, kt, cc0:cc1], ev_l,
                                total=VSH, chunks=(CH[ci],))

    nc.compile()
    return nc


def _part(x, kt):
    """[R, C] -> [128, R//128, C] with row = kt_idx*128 + p."""
    R, C = x.shape
    return np.ascontiguousarray(x.reshape(kt, P, C).transpose(1, 0, 2))


def kernel(**inputs):
    inp = {k: np.asarray(v) for k, v in inputs.items()}
    ids = inp["input_ids"].astype(np.int64)[0]          # [S]
    memory = inp["memory"].astype(np.float32)[0]        # [MLEN, DM]
    f = np.float32

    ln1 = inp["ln1"].astype(f)
    lnc = inp["lnc"].astype(f)
    ln2 = inp["ln2"].astype(f)
    lnf = inp["lnf"].astype(f)

    h0 = inp["embed"].astype(f)[ids]                    # [S, D]
    hT0 = _part(h0.T.astype(np.float16), DKT)           # [128, 32, S]
    memT = _part(memory.T.astype(np.float16), DMKT)     # [128, 8, MLEN]

    # RoPE tables (transposed layout [DH, S])
    inv = 1.0 / (10000.0 ** (np.arange(0, DH, 2, dtype=f) / DH))
    t = np.arange(S, dtype=f)
    freqs = np.outer(t, inv)                            # [S, DH//2]
    emb = np.concatenate([freqs, freqs], axis=1)        # [S, DH]
    cosT = np.cos(emb).T.astype(np.float16)             # [DH, S]
    sinT = np.sin(emb).T.astype(np.float16)
    rotM = np.zeros((P, P), dtype=np.float16)           # rotM[k,d]: rot_half
    rotM[np.arange(64) + 64, np.arange(64)] = -1.0      # out[d<64] = -in[d+64]
    rotM[np.arange(64), np.arange(64) + 64] = 1.0       # out[d>=64] = in[d-64]
    maskT = np.triu(np.ones((P, P), dtype=np.float16))  # [key p, query col]

    wq = inp["wq"].astype(f) * ln1[:, None]
    wk = inp["wk"].astype(f) * ln1[:, None]
    wv = inp["wv"].astype(f) * ln1[:, None]
    cwq = inp["cwq"].astype(f) * lnc[:, None]
    cwk = inp["cwk"].astype(f)
    cwv = inp["cwv"].astype(f)
    wg = inp["wg"].astype(f) * ln2[:, None]
    wu = inp["wu"].astype(f) * ln2[:, None]
    lmh = inp["lm_head"].astype(f) * lnf[:, None]
    wo = inp["wo"].astype(f)
    cwo = inp["cwo"].astype(f)
    wd = inp["wd"].astype(f)
    pw1 = inp["p_w1"].astype(f)
    pw2 = inp["p_w2"].astype(f)
    pb1 = inp["p_b1"].astype(f)
    pb2 = inp["p_b2"].astype(f)

    h16 = np.float16
    in_maps = []
    for c in range(NCORES):
        ds = slice(c * DSH, (c + 1) * DSH)
        ffs = slice(c * FFSH, (c + 1) * FFSH)
        phs = slice(c * PHS, (c + 1) * PHS)
        vs = slice(c * VSH, (c + 1) * VSH)

        # gate/up column tiles interleaved: g0,u0,g1,u1,...
        g_pad = np.zeros((D, FFPAD), dtype=h16)
        g_pad[:, 0:FFSH] = wg[:, ffs].astype(h16)
        u_pad = np.zeros((D, FFPAD), dtype=h16)
        u_pad[:, 0:FFSH] = wu[:, ffs].astype(h16)
        wgu_c = np.empty((D, 2 * FFPAD), dtype=h16)
        for ft in range(FFKT):
            wgu_c[:, (2 * ft) * P:(2 * ft + 1) * P] = \
                g_pad[:, ft * P:(ft + 1) * P]
            wgu_c[:, (2 * ft + 1) * P:(2 * ft + 2) * P] = \
                u_pad[:, ft * P:(ft + 1) * P]
        wd_c = np.zeros((FFPAD, D), dtype=h16)
        wd_c[0:FFSH] = wd[ffs, :].astype(h16)

        m = {
            "hT0": hT0, "memT": memT,
            "pw1": _part(pw1[:, phs].astype(h16), DMKT),
            "pw2": _part(pw2[phs, :].astype(h16), PHKT),
            "pb1": np.ascontiguousarray(pb1[phs].reshape(PHKT, P).T.astype(f)),
            "pb2": np.ascontiguousarray((pb2 / NCORES).reshape(DKT, P).T.astype(f)),
            "wqk": _part(np.concatenate([wq[:, ds], wk[:, ds]], axis=1).astype(h16), DKT),
            "wv": _part(wv[:, ds].astype(h16), DKT),
            "wo": _part(wo[ds, :].astype(h16), DSH // P),
            "cwqk": _part(np.concatenate([cwq[:, ds], cwk[:, ds]], axis=1).astype(h16), DKT),
            "cwv": _part(cwv[:, ds].astype(h16), DKT),
            "cwo": _part(cwo[ds, :].astype(h16), DSH // P),
            "wgu": _part(wgu_c, DKT),
            "wd": _part(wd_c, FFKT),
            "lmh": _part(lmh[:, vs].astype(h16), DKT),
            "cosT": cosT, "sinT": sinT, "rotM": rotM, "maskT": maskT,
        }
        in_maps.append(m)

    if "nc" not in _prog_cache:
        _prog_cache["nc"] = _build_program()
    nc = _prog_cache["nc"]

    res = run_bass_kernel_spmd(nc, in_maps, list(range(NCORES)))
    global LAST_RESULTS
    LAST_RESULTS = res
    logits = np.concatenate([r["logitsT"].T for r in res.results], axis=1)
    return logits.reshape(B, S, V).astype(np.float32)


if __name__ == "__main__":
    # quick build check
    nc = _build_program()
    print("program built ok")


# revision 32
# speedup vs baseline: 1.1746x; 1.0164x over previous
# Trainium2 Bass kernel for nn_Decoder_51582557225714.
# 8-way tensor-parallel single-layer decoder with cross-attention.
#
# Sharding (per core c of 8):
#  - q/k/v/o, cross q/k/v/o: column-shard by head (4 heads = 512 cols per core),
#    o/cwo row-sharded; partial outputs AllReduced.
#  - MLP gate/up column-shard (1376 -> padded 1408 cols), down row-shard, AllReduce.
#  - projector: p_w1 column-shard (1024 cols of PH), p_w2 row-shard, AllReduce.
#  - lm_head vocab-shard (1000 cols per core), gathered on host.
#  - embedding gather + all input sharding/transposition done host-side.
#
# v2 schedule (keeps TensorE dense + overlaps every collective):
#  - projector runs first (its matmuls warm the PE while hT0 uploads);
#    mem AllReduce rides under block0's projections.
#  - projected memory is AllReduced once and then kept resident in SBUF, so
#    cross-attention k/v projections stream at full rate (no HBM re-reads).
#  - cross k/v projections are emitted right after block0's o-projection so
#    they execute during block0's residual AllReduce.
#  - every residual AllReduce is split into two S-halves; producers evict
#    chunk-major so the first half reduces while the second is computed, and
#    consumers (norm + next projection) start on half 0 while half 1 reduces.
#  - attention is causally trimmed (no matmuls/exp on the all-zero triangle)
#    and processed chunk-major with per-kt probability tiles.
#  - swiglu is fused into the gate/up evictions (wgu tiles interleaved g,u).
# All activations kept TRANSPOSED ([feature, seq]) on device; fp16 data with
# fp32 PSUM accumulation; rmsnorm folded into weights (ln scale) + column
# rescale (rsqrt); softmax without max-subtraction, exp(score-5).

import math
import numpy as np

import concourse.bass as bass
import concourse.mybir as mybir
import concourse.tile as tile
from concourse import bacc
from concourse.bass_utils import run_bass_kernel_spmd

P = 128
NCORES = 8
B, S, MLEN = 1, 1024, 1024
D, H, DH, FF = 4096, 32, 128, 11008
V, DM, PH = 8000, 1024, 8192
EPS = 1e-6

DKT = D // P            # 32 k-tiles over D
DMKT = DM // P          # 8
HSH = H // NCORES       # 4 heads per core
DSH = HSH * DH          # 512
FFSH = FF // NCORES     # 1376
FFPAD = 1408            # padded to 11*128
FFKT = FFPAD // P       # 11
PHS = PH // NCORES      # 1024
PHKT = PHS // P         # 8
VSH = V // NCORES       # 1000
SKT = S // P            # 8
CH = ((0, 512), (512, 1024))

f32 = mybir.dt.float32
f16 = mybir.dt.float16
AF = mybir.ActivationFunctionType
ALU = mybir.AluOpType

_prog_cache = {}
LAST_RESULTS = None  # BassKernelResults of the most recent run (for harness use)


def _build_program():
    nc = bacc.Bacc("TRN2", target_bir_lowering=False, debug=False,
                   enable_asserts=True, num_devices=NCORES)

    # ---- I/O declarations (per core) ----
    def din(name, shape, dt=f16):
        return nc.dram_tensor(name, shape, dt, kind="ExternalInput")

    hT0_d = din("hT0", [P, DKT, S])
    memT_d = din("memT", [P, DMKT, MLEN])
    pw1_d = din("pw1", [P, DMKT, PHS])
    pw2_d = din("pw2", [P, PHKT, D])
    pb1_d = din("pb1", [P, PHKT], f32)
    pb2_d = din("pb2", [P, DKT], f32)          # p_b2 / 8
    wqk_d = din("wqk", [P, DKT, 2 * DSH])
    wv_d = din("wv", [P, DKT, DSH])
    wo_d = din("wo", [P, DSH // P, D])
    cwqk_d = din("cwqk", [P, DKT, 2 * DSH])
    cwv_d = din("cwv", [P, DKT, DSH])
    cwo_d = din("cwo", [P, DSH // P, D])
    wgu_d = din("wgu", [P, DKT, 2 * FFPAD])    # tiles interleaved g0,u0,g1,u1,...
    wd_d = din("wd", [P, FFKT, D])
    lmh_d = din("lmh", [P, DKT, VSH])
    cosT_d = din("cosT", [P, S])
    sinT_d = din("sinT", [P, S])
    rotM_d = din("rotM", [P, P])
    maskT_d = din("maskT", [P, P])

    logits_d = nc.dram_tensor("logitsT", [VSH, S], f32, kind="ExternalOutput")

    # collective bounce buffers
    mem_par = nc.dram_tensor("mem_par", [P, DKT, MLEN], f16)
    mem_red = nc.dram_tensor("mem_red", [P, DKT, MLEN], f16, addr_space="Shared")
    blk_par = [[nc.dram_tensor(f"blk_par{i}_{c}", [P, DKT, 512], f16)
                for c in range(2)] for i in range(3)]
    blk_red = [[nc.dram_tensor(f"blk_red{i}_{c}", [P, DKT, 512], f16,
                               addr_space="Shared")
                for c in range(2)] for i in range(3)]
    scratch_rs = nc.dram_tensor("rs_scratch", [S], f32)

    rg = [list(range(NCORES))]

    with tile.TileContext(nc) as tc:
        with (
            tc.tile_pool(name="persist", bufs=1) as persist,
            # Global scratch pools, allocated once for the whole kernel.
            # Phase-local pools land in just-released SBUF zones and their
            # first weight DMA waits on every reader of the previous phase
            # (zone-overlap dep) — persistent pools give per-slot deps, so
            # weight prefetch crosses phase boundaries.
            tc.tile_pool(name="gw32", bufs=2) as gw32,
            tc.tile_pool(name="gw11", bufs=3) as gw11,
            tc.tile_pool(name="gw8", bufs=2) as gw8,
            tc.tile_pool(name="gw4", bufs=4) as gw4,
            tc.tile_pool(name="gev", bufs=3) as gev,
            tc.tile_pool(name="gsq", bufs=2) as gsq,
            tc.tile_pool(name="gat", bufs=4) as gat,
            tc.tile_pool(name="gat2", bufs=2) as gat2,
            tc.tile_pool(name="gwv", bufs=3) as gwv,
        ):
            WPOOL = {32: gw32, 11: gw11, 8: gw8, 4: gw4}
            hT = persist.tile([P, DKT, S], f16)
            cosT = persist.tile([P, S], f16)
            sinT = persist.tile([P, S], f16)
            rotM = persist.tile([P, P], f16)
            maskT = persist.tile([P, P], f16)
            ones = persist.tile([P, 1], f16)
            nc.sync.dma_start(out=cosT[:], in_=cosT_d[:])
            nc.sync.dma_start(out=sinT[:], in_=sinT_d[:])
            nc.sync.dma_start(out=rotM[:], in_=rotM_d[:])
            nc.sync.dma_start(out=maskT[:], in_=maskT_d[:])
            nc.vector.memset(ones[:], 1.0)
            eps_t = persist.tile([1, 1], f32)
            nc.vector.memset(eps_t[:], EPS)
            nexp_t = persist.tile([P, 1], f32)
            nc.vector.memset(nexp_t[:], -5.0)

            # ---------- shared emitters ----------
            def emit_norm(nm, pool, c0, c1, want_q=False, want_t=False,
                          sq_dve=False):
                """rsqrt(mean(h^2)+eps) over cols [c0,c1). Returns
                (rbc [P,w] f32, rbcq or None, rT [P,SKT] f32 or None)."""
                w = c1 - c0
                with tc.tile_pool(name=f"{nm}_sp", bufs=1,
                                  space="PSUM") as sps:
                    ps = sps.tile([1, w], f32)
                    for kt in range(DKT):
                        hsq = gsq.tile([P, w], f16, tag="hsq",
                                       name=f"hsq_{nm}_{kt}")
                        src = hT[:, kt, c0:c1]
                        # split the squares across ACT and DVE
                        if sq_dve or kt % 2 == 1:
                            nc.vector.tensor_mul(hsq[:], src, src)
                        else:
                            nc.scalar.activation(hsq[:], src, AF.Square)
                        # one psum bank (512 f32) per matmul
                        for s0 in range(0, w, 512):
                            s1 = min(w, s0 + 512)
                            nc.tensor.matmul(ps[0:1, s0:s1], ones[:, 0:1],
                                             hsq[:, s0:s1],
                                             start=(kt == 0),
                                             stop=(kt == DKT - 1))
                    row = gsq.tile([1, w], f32, tag="row", name=f"row_{nm}")
                    nc.scalar.activation(row[:], ps[0:1, :], AF.Sqrt,
                                         scale=1.0 / D, bias=eps_t[0:1, 0:1])
                    rrow = gsq.tile([1, w], f32, tag="rrow", name=f"rrow_{nm}")
                    nc.vector.reciprocal(rrow[:], row[:])

                    rbc = pool.tile([P, w], f32, tag=f"{nm}_rbc")
                    nc.gpsimd.partition_broadcast(rbc[:], rrow[0:1, :])
                    rbcq = None
                    if want_q:
                        rbcq = pool.tile([P, w], f32, tag=f"{nm}_rbcq")
                        nc.vector.tensor_scalar_mul(rbcq[:], rbc[:],
                                                    1.0 / math.sqrt(DH))
                    rT = None
                    if want_t:
                        nc.sync.dma_start(out=scratch_rs[:], in_=rrow[0:1, :])
                        rT = pool.tile([P, SKT], f32, tag=f"{nm}_rT")
                        nc.sync.dma_start(
                            out=rT[:],
                            in_=scratch_rs.ap().rearrange("(kt p) -> p kt", p=P))
                return rbc, rbcq, rT

            def emit_attention(nm, qkT, v_sb, attn_o):
                """Causally-trimmed attention, chunk-major over q columns.
                qkT [P, 2*HSH, S] (q tiles 0..HSH-1 scaled, k tiles HSH..).
                v_sb [P, SKT, DSH]. attn_o [P, HSH, S] f16."""
                with (
                    tc.tile_pool(name=f"{nm}_ps", bufs=2, space="PSUM") as psp,
                    tc.tile_pool(name=f"{nm}_po", bufs=2, space="PSUM") as pop,
                    tc.tile_pool(name=f"{nm}_pc", bufs=2, space="PSUM") as pcp,
                ):
                    for ci, (c0, c1) in enumerate(CH):
                        kt_last = c1 // P - 1
                        for h in range(HSH):
                            ps_o = pop.tile([P, 512], f32, tag="ps_o")
                            ps_cs = pcp.tile([1, 512], f32, tag="ps_cs")
                            for kt in range(SKT):
                                n0 = kt * P
                                a0 = max(c0, n0)
                                if a0 >= c1:
                                    continue
                                w = c1 - a0
                                ps_s = psp.tile([P, 512], f32, tag="ps_s")
                                nc.tensor.matmul(
                                    ps_s[:, 0:w], qkT[:, HSH + h, n0:n0 + P],
                                    qkT[:, h, a0:c1], start=True, stop=True)
                                pt = gat.tile([P, w], f16, tag="pt",
                                              name=f"pt_{nm}_{ci}_{kt}_{h}")
                                nc.scalar.activation(pt[:], ps_s[:, 0:w], AF.Exp,
                                                     bias=nexp_t[:, 0:1])
                                if a0 == n0:
                                    nc.vector.tensor_mul(pt[:, 0:P], pt[:, 0:P],
                                                         maskT[:])
                                st = (kt == 0)
                                sp = (kt == kt_last)
                                nc.tensor.matmul(ps_cs[0:1, a0 - c0:c1 - c0],
                                                 ones[:, 0:1], pt[:],
                                                 start=st, stop=sp)
                                nc.tensor.matmul(
                                    ps_o[:, a0 - c0:c1 - c0],
                                    v_sb[:, kt, h * DH:(h + 1) * DH], pt[:],
                                    start=st, stop=sp)
                            rrow = gat2.tile([1, 512], f32, tag="rrow",
                                             name=f"arow_{nm}_{ci}_{h}")
                            nc.vector.reciprocal(rrow[:], ps_cs[0:1, :])
                            rbc = gat2.tile([P, 512], f32, tag="rbc",
                                            name=f"abc_{nm}_{ci}_{h}")
                            nc.gpsimd.partition_broadcast(rbc[:], rrow[0:1, :])
                            nc.vector.tensor_mul(attn_o[:, h, c0:c1], ps_o[:],
                                                 rbc[:])

            def stream_proj(nm, w_dram, nmt, nkt, rhs_fn, evict_fn,
                            chunks=CH, wbufs=2, total=None):
                """Chunk-major weight-streaming projection.
                out[mt, c0:c1] = sum_kt w[:, kt, mslice].T @ rhs(kt, c0, c1).
                evict_fn(c0, c1, mt, ps, mw) consumes psum [mw, c1-c0].
                Weight tiles come from the persistent per-nkt pools."""
                if total is None:
                    total = nmt * P
                wp = WPOOL[nkt]
                with tc.tile_pool(name=f"{nm}_p", bufs=2, space="PSUM") as pp:
                    for c0, c1 in chunks:
                        for mt in range(nmt):
                            m0 = mt * P
                            mw = min(P, total - m0)
                            wt = wp.tile([P, nkt, P], f16, tag=f"wt{nkt}",
                                         name=f"wt_{nm}_{c0}_{mt}")
                            nc.sync.dma_start(out=wt[:, :, 0:mw],
                                              in_=w_dram[:, :, m0:m0 + mw])
                            ps = pp.tile([P, 512], f32, tag="ps")
                            for kt in range(nkt):
                                nc.tensor.matmul(ps[0:mw, 0:c1 - c0],
                                                 wt[:, kt, 0:mw],
                                                 rhs_fn(kt, c0, c1),
                                                 start=(kt == 0),
                                                 stop=(kt == nkt - 1))
                            evict_fn(c0, c1, mt, ps, mw)

            # ================= projector =================
            with tc.tile_pool(name="proj", bufs=1) as projp:
                memT_sb = projp.tile([P, DMKT, MLEN], f16)
                nc.sync.dma_start(out=memT_sb[:], in_=memT_d[:])
                pb1_sb = projp.tile([P, PHKT], f32)
                pb2_sb = projp.tile([P, DKT], f32)
                nc.sync.dma_start(out=pb1_sb[:], in_=pb1_d[:])
                nc.sync.dma_start(out=pb2_sb[:], in_=pb2_d[:])
                gT = projp.tile([P, PHKT, MLEN], f16)

                def ev_g(c0, c1, mt, ps, mw):
                    nc.scalar.activation(gT[:, mt, c0:c1], ps[:], AF.Gelu,
                                         bias=pb1_sb[:, mt:mt + 1])
                stream_proj("pj1", pw1_d, PHKT, DMKT,
                            lambda kt, c0, c1: memT_sb[:, kt, c0:c1], ev_g)

                # hidden-state upload rides behind the projector inputs in
                # DMA-lane order; it is only needed ~100us later
                for q in range(4):
                    nc.sync.dma_start(out=hT[:, 8 * q:8 * q + 8, :],
                                      in_=hT0_d[:, 8 * q:8 * q + 8, :])

                def ev_m(c0, c1, mt, ps, mw):
                    t = gev.tile([P, 512], f16, tag="ev16",
                                 name=f"mev_{c0}_{mt}")
                    nc.scalar.activation(t[:], ps[:], AF.Identity,
                                         bias=pb2_sb[:, mt:mt + 1])
                    nc.sync.dma_start(out=mem_par[:, mt, c0:c1], in_=t[:])
                stream_proj("pj2", pw2_d, DKT, PHKT,
                            lambda kt, c0, c1: gT[:, kt, c0:c1], ev_m)

                nc.gpsimd.collective_compute(
                    "AllReduce", ALU.add, ins=[mem_par[:]], outs=[mem_red[:]],
                    replica_groups=rg)

            # ================= block0: self-attention =================
            with (
                tc.tile_pool(name="b0norm", bufs=1) as b0_np,
                tc.tile_pool(name="b0act", bufs=1) as b0_act,
            ):
                qkT0 = b0_act.tile([P, 2 * HSH, S], f16)
                v0_sb = b0_act.tile([P, SKT, DSH], f16)

                def ev_qk0(c0, c1, mt, ps, mw):
                    nc.scalar.activation(qkT0[:, mt, c0:c1], ps[:], AF.Copy)
                stream_proj("b0qk", wqk_d, 2 * HSH, DKT,
                            lambda kt, c0, c1: hT[:, kt, c0:c1], ev_qk0)

                rbc0, rbcq0, rT0 = emit_norm("n0", b0_np, 0, S,
                                             want_q=True, want_t=True,
                                             sq_dve=True)

                # v projection: lhsT = hT seq slices, rhs = wv tiles
                with tc.tile_pool(name="b0vp", bufs=1, space="PSUM") as vps:
                    for half in range(2):
                        pss = [vps.tile([P, DSH], f32, tag=f"psv{i}",
                                        name=f"psv0_{half}_{i}")
                               for i in range(4)]
                        for kt in range(DKT):
                            wvt = gwv.tile([P, DSH], f16, tag="wvt",
                                           name=f"wvt0_{half}_{kt}")
                            nc.sync.dma_start(out=wvt[:], in_=wv_d[:, kt, :])
                            for i in range(4):
                                mt = half * 4 + i
                                nc.tensor.matmul(
                                    pss[i][:], hT[:, kt, mt * P:(mt + 1) * P],
                                    wvt[:], start=(kt == 0),
                                    stop=(kt == DKT - 1))
                        for i in range(4):
                            mt = half * 4 + i
                            nc.scalar.activation(v0_sb[:, mt, :], pss[i][:],
                                                 AF.Copy,
                                                 scale=rT0[:, mt:mt + 1])

                # rope on q and k tiles + norm/softmax scaling
                with (
                    tc.tile_pool(name="b0r", bufs=2) as rp,
                    tc.tile_pool(name="b0rp", bufs=2, space="PSUM") as rps,
                ):
                    for t in range(2 * HSH):
                        sc = rbcq0 if t < HSH else rbc0
                        for c0, c1 in CH:
                            psr = rps.tile([P, 512], f32, tag="psr")
                            nc.tensor.matmul(psr[:], rotM[:], qkT0[:, t, c0:c1],
                                             start=True, stop=True)
                            t2 = rp.tile([P, 512], f16, tag="t2")
                            nc.vector.tensor_mul(t2[:], psr[:], sinT[:, c0:c1])
                            t3 = rp.tile([P, 512], f16, tag="t3")
                            nc.vector.tensor_mul(t3[:], qkT0[:, t, c0:c1],
                                                 cosT[:, c0:c1])
                            nc.vector.tensor_add(t2[:], t2[:], t3[:])
                            nc.vector.tensor_mul(qkT0[:, t, c0:c1], t2[:],
                                                 sc[:, c0:c1])

                # attention outputs overwrite the q slots of qkT0 (each
                # write touches only columns whose scores are already done)
                emit_attention("a0", qkT0, v0_sb, qkT0)

                # o-projection, chunk-major; AllReduce per half
                with tc.tile_pool(name="b0o_p", bufs=2, space="PSUM") as pp0:
                    for ci, (c0, c1) in enumerate(CH):
                        for mt in range(DKT):
                            wt = gw4.tile([P, HSH, P], f16, tag="wt4",
                                          name=f"wo0_{ci}_{mt}")
                            nc.sync.dma_start(
                                out=wt[:],
                                in_=wo_d[:, :, mt * P:(mt + 1) * P])
                            ps = pp0.tile([P, 512], f32, tag="ps")
                            for kt in range(HSH):
                                nc.tensor.matmul(ps[:], wt[:, kt, :],
                                                 qkT0[:, kt, c0:c1],
                                                 start=(kt == 0),
                                                 stop=(kt == HSH - 1))
                            t_ = gev.tile([P, 512], f16, tag="ev16",
                                          name=f"oev0_{ci}_{mt}")
                            nc.vector.scalar_tensor_tensor(
                                t_[:], hT[:, mt, c0:c1], 1.0 / NCORES,
                                ps[:], ALU.mult, ALU.add)
                            nc.sync.dma_start(out=blk_par[0][ci][:, mt, :],
                                              in_=t_[:])
                        nc.gpsimd.collective_compute(
                            "AllReduce", ALU.add, ins=[blk_par[0][ci][:]],
                            outs=[blk_red[0][ci][:]], replica_groups=rg)

            # ===== block1 k/v projections (from SBUF-resident memory) =====
            # these run during block0's residual AllReduce
            with tc.tile_pool(name="b1act", bufs=1) as b1_act:
                qkT1 = b1_act.tile([P, 2 * HSH, S], f16)
                v1_sb = b1_act.tile([P, SKT, DSH], f16)

                # memory processed in two column halves to halve SBUF
                # residency; each half feeds both the k columns and the
                # v seq-tiles that live in those columns.
                with (
                    tc.tile_pool(name="memr", bufs=1) as memrp,
                    tc.tile_pool(name="b1vp", bufs=1, space="PSUM") as vps1,
                ):
                    for ci, (c0, c1) in enumerate(CH):
                        memR = memrp.tile([P, DKT, 512], f16, tag="memR",
                                          name=f"memR{ci}")
                        nc.sync.dma_start(out=memR[:],
                                          in_=mem_red.ap()[:, :, c0:c1])

                        def ev_k1(cc0, cc1, mt, ps, mw):
                            nc.scalar.activation(qkT1[:, HSH + mt, cc0:cc1],
                                                 ps[:], AF.Copy)
                        stream_proj(f"b1k{ci}",
                                    cwqk_d.ap()[:, :, DSH:2 * DSH], HSH, DKT,
                                    lambda kt, cc0, cc1:
                                        memR[:, kt, 0:cc1 - cc0],
                                    ev_k1, chunks=(CH[ci],))

                        pss = [vps1.tile([P, DSH], f32, tag=f"psv{i}",
                                         name=f"psv1_{ci}_{i}")
                               for i in range(4)]
                        for kt in range(DKT):
                            wvt = gwv.tile([P, DSH], f16, tag="wvt",
                                           name=f"wvt1_{ci}_{kt}")
                            nc.sync.dma_start(out=wvt[:], in_=cwv_d[:, kt, :])
                            for i in range(4):
                                nc.tensor.matmul(
                                    pss[i][:],
                                    memR[:, kt, i * P:(i + 1) * P],
                                    wvt[:], start=(kt == 0),
                                    stop=(kt == DKT - 1))
                        for i in range(4):
                            mt = 4 * ci + i
                            nc.scalar.activation(v1_sb[:, mt, :], pss[i][:],
                                                 AF.Copy)

                # ===== reload hT halves; norm1 + q1, per chunk so chunk-a
                # work never queues behind chunk-b dependencies =====
                with tc.tile_pool(name="b1norm", bufs=1) as b1_np:
                    for ci, (c0, c1) in enumerate(CH):
                        nc.sync.dma_start(out=hT[:, :, c0:c1],
                                          in_=blk_red[0][ci][:])
                        _, rbcq1, _ = emit_norm(f"n1{ci}", b1_np, c0, c1,
                                                want_q=True)

                        def ev_q1(cc0, cc1, mt, ps, mw, rb=rbcq1):
                            nc.vector.tensor_mul(qkT1[:, mt, cc0:cc1], ps[:],
                                                 rb[:])
                        stream_proj(f"b1q{ci}", cwqk_d.ap()[:, :, 0:DSH],
                                    HSH, DKT,
                                    lambda kt, cc0, cc1: hT[:, kt, cc0:cc1],
                                    ev_q1, chunks=(CH[ci],))

                    emit_attention("a1", qkT1, v1_sb, qkT1)

                    # o-projection, chunk-major; AllReduce per half
                    with tc.tile_pool(name="b1o_p", bufs=2,
                                      space="PSUM") as pp1:
                        for ci, (c0, c1) in enumerate(CH):
                            for mt in range(DKT):
                                wt = gw4.tile([P, HSH, P], f16, tag="wt4",
                                              name=f"wo1_{ci}_{mt}")
                                nc.sync.dma_start(
                                    out=wt[:],
                                    in_=cwo_d[:, :, mt * P:(mt + 1) * P])
                                ps = pp1.tile([P, 512], f32, tag="ps")
                                for kt in range(HSH):
                                    nc.tensor.matmul(ps[:], wt[:, kt, :],
                                                     qkT1[:, kt, c0:c1],
                                                     start=(kt == 0),
                                                     stop=(kt == HSH - 1))
                                t_ = gev.tile([P, 512], f16, tag="ev16",
                                              name=f"oev1_{ci}_{mt}")
                                nc.vector.scalar_tensor_tensor(
                                    t_[:], hT[:, mt, c0:c1], 1.0 / NCORES,
                                    ps[:], ALU.mult, ALU.add)
                                nc.sync.dma_start(out=blk_par[1][ci][:, mt, :],
                                                  in_=t_[:])
                            nc.gpsimd.collective_compute(
                                "AllReduce", ALU.add, ins=[blk_par[1][ci][:]],
                                outs=[blk_red[1][ci][:]], replica_groups=rg)

            # ================= MLP (swiglu fused into evictions) ============
            with (
                tc.tile_pool(name="mlpnorm", bufs=1) as mlp_np,
                tc.tile_pool(name="mlpact", bufs=1) as mlp_act,
            ):
                guT = mlp_act.tile([P, FFKT, S], f16)
                sg_t = {}
                for ci, (c0, c1) in enumerate(CH):
                    nc.sync.dma_start(out=hT[:, :, c0:c1], in_=blk_red[1][ci][:])
                    rbc2, _, _ = emit_norm(f"n2{ci}", mlp_np, c0, c1)

                    def ev_gu(cc0, cc1, mt, ps, mw, rb=rbc2):
                        ft = mt // 2
                        if mt % 2 == 0:     # gate tile: silu(g * rbc2)
                            gs = gev.tile([P, 512], f16, tag="ev16",
                                          name=f"gs_{cc0}_{ft}")
                            nc.vector.tensor_mul(gs[:], ps[:], rb[:])
                            sg = gev.tile([P, 512], f16, tag="sg",
                                          name=f"sg_{cc0}_{ft}")
                            nc.scalar.activation(sg[:], gs[:], AF.Silu)
                            sg_t[(cc0, ft)] = sg
                        else:               # up tile: (u * rbc2) * silu_gate
                            us = gev.tile([P, 512], f16, tag="ev16",
                                          name=f"us_{cc0}_{ft}")
                            nc.vector.tensor_mul(us[:], ps[:], rb[:])
                            nc.vector.tensor_mul(guT[:, ft, cc0:cc1],
                                                 sg_t.pop((cc0, ft))[:], us[:])
                    stream_proj(f"mgu{ci}", wgu_d, 2 * FFKT, DKT,
                                lambda kt, cc0, cc1: hT[:, kt, cc0:cc1],
                                ev_gu, chunks=(CH[ci],))

                # down projection, chunk-major; AllReduce per half
                with tc.tile_pool(name="md_p", bufs=2, space="PSUM") as ppd:
                    for ci, (c0, c1) in enumerate(CH):
                        for mt in range(DKT):
                            wt = gw11.tile([P, FFKT, P], f16, tag="wt11",
                                           name=f"wd_{ci}_{mt}")
                            nc.sync.dma_start(
                                out=wt[:], in_=wd_d[:, :, mt * P:(mt + 1) * P])
                            ps = ppd.tile([P, 512], f32, tag="ps")
                            for kt in range(FFKT):
                                nc.tensor.matmul(ps[:], wt[:, kt, :],
                                                 guT[:, kt, c0:c1],
                                                 start=(kt == 0),
                                                 stop=(kt == FFKT - 1))
                            t_ = gev.tile([P, 512], f16, tag="ev16",
                                          name=f"dev_{ci}_{mt}")
                            nc.vector.scalar_tensor_tensor(
                                t_[:], hT[:, mt, c0:c1], 1.0 / NCORES, ps[:],
                                ALU.mult, ALU.add)
                            nc.sync.dma_start(out=blk_par[2][ci][:, mt, :],
                                              in_=t_[:])
                        nc.gpsimd.collective_compute(
                            "AllReduce", ALU.add, ins=[blk_par[2][ci][:]],
                            outs=[blk_red[2][ci][:]], replica_groups=rg)

            # ================= lm head =================
            with tc.tile_pool(name="lmhnorm", bufs=1) as lmh_np:
                for ci, (c0, c1) in enumerate(CH):
                    nc.sync.dma_start(out=hT[:, :, c0:c1], in_=blk_red[2][ci][:])
                    rbc3, _, _ = emit_norm(f"n3{ci}", lmh_np, c0, c1)

                    def ev_l(cc0, cc1, mt, ps, mw, rb=rbc3):
                        t_ = gev.tile([P, 512], f32, tag="lev",
                                      name=f"lev_{cc0}_{mt}")
                        nc.vector.tensor_mul(t_[0:mw, :], ps[0:mw, :],
                                             rb[0:mw, :])
                        nc.sync.dma_start(
                            out=logits_d.ap()[mt * P:mt * P + mw, cc0:cc1],
                            in_=t_[0:mw, :])
                    stream_proj(f"lh{ci}", lmh_d, (VSH + P - 1) // P, DKT,
                                lambda kt, cc0, cc1: hT[:, kt, cc0:cc1], ev_l,
                                total=VSH, chunks=(CH[ci],))

    nc.compile()
    return nc


def _part(x, kt):
    """[R, C] -> [128, R//128, C] with row = kt_idx*128 + p."""
    R, C = x.shape
    return np.ascontiguousarray(x.reshape(kt, P, C).transpose(1, 0, 2))


def kernel(**inputs):
    inp = {k: np.asarray(v) for k, v in inputs.items()}
    ids = inp["input_ids"].astype(np.int64)[0]          # [S]
    memory = inp["memory"].astype(np.float32)[0]        # [MLEN, DM]
    f = np.float32

    ln1 = inp["ln1"].astype(f)
    lnc = inp["lnc"].astype(f)
    ln2 = inp["ln2"].astype(f)
    lnf = inp["lnf"].astype(f)

    h0 = inp["embed"].astype(f)[ids]                    # [S, D]
    hT0 = _part(h0.T.astype(np.float16), DKT)           # [128, 32, S]
    memT = _part(memory.T.astype(np.float16), DMKT)     # [128, 8, MLEN]

    # RoPE tables (transposed layout [DH, S])
    inv = 1.0 / (10000.0 ** (np.arange(0, DH, 2, dtype=f) / DH))
    t = np.arange(S, dtype=f)
    freqs = np.outer(t, inv)                            # [S, DH//2]
    emb = np.concatenate([freqs, freqs], axis=1)        # [S, DH]
    cosT = np.cos(emb).T.astype(np.float16)             # [DH, S]
    sinT = np.sin(emb).T.astype(np.float16)
    rotM = np.zeros((P, P), dtype=np.float16)           # rotM[k,d]: rot_half
    rotM[np.arange(64) + 64, np.arange(64)] = -1.0      # out[d<64] = -in[d+64]
    rotM[np.arange(64), np.arange(64) + 64] = 1.0       # out[d>=64] = in[d-64]
    maskT = np.triu(np.ones((P, P), dtype=np.float16))  # [key p, query col]

    wq = inp["wq"].astype(f) * ln1[:, None]
    wk = inp["wk"].astype(f) * ln1[:, None]
    wv = inp["wv"].astype(f) * ln1[:, None]
    cwq = inp["cwq"].astype(f) * lnc[:, None]
    cwk = inp["cwk"].astype(f)
    cwv = inp["cwv"].astype(f)
    wg = inp["wg"].astype(f) * ln2[:, None]
    wu = inp["wu"].astype(f) * ln2[:, None]
    lmh = inp["lm_head"].astype(f) * lnf[:, None]
    wo = inp["wo"].astype(f)
    cwo = inp["cwo"].astype(f)
    wd = inp["wd"].astype(f)
    pw1 = inp["p_w1"].astype(f)
    pw2 = inp["p_w2"].astype(f)
    pb1 = inp["p_b1"].astype(f)
    pb2 = inp["p_b2"].astype(f)

    h16 = np.float16
    in_maps = []
    for c in range(NCORES):
        ds = slice(c * DSH, (c + 1) * DSH)
        ffs = slice(c * FFSH, (c + 1) * FFSH)
        phs = slice(c * PHS, (c + 1) * PHS)
        vs = slice(c * VSH, (c + 1) * VSH)

        # gate/up column tiles interleaved: g0,u0,g1,u1,...
        g_pad = np.zeros((D, FFPAD), dtype=h16)
        g_pad[:, 0:FFSH] = wg[:, ffs].astype(h16)
        u_pad = np.zeros((D, FFPAD), dtype=h16)
        u_pad[:, 0:FFSH] = wu[:, ffs].astype(h16)
        wgu_c = np.empty((D, 2 * FFPAD), dtype=h16)
        for ft in range(FFKT):
            wgu_c[:, (2 * ft) * P:(2 * ft + 1) * P] = \
                g_pad[:, ft * P:(ft + 1) * P]
            wgu_c[:, (2 * ft + 1) * P:(2 * ft + 2) * P] = \
                u_pad[:, ft * P:(ft + 1) * P]
        wd_c = np.zeros((FFPAD, D), dtype=h16)
        wd_c[0:FFSH] = wd[ffs, :].astype(h16)

        m = {
            "hT0": hT0, "memT": memT,
            "pw1": _part(pw1[:, phs].astype(h16), DMKT),
            "pw2": _part(pw2[phs, :].astype(h16), PHKT),
            "pb1": np.ascontiguousarray(pb1[phs].reshape(PHKT, P).T.astype(f)),
            "pb2": np.ascontiguousarray((pb2 / NCORES).reshape(DKT, P).T.astype(f)),
            "wqk": _part(np.concatenate([wq[:, ds], wk[:, ds]], axis=1).astype(h16), DKT),
            "wv": _part(wv[:, ds].astype(h16), DKT),
            "wo": _part(wo[ds, :].astype(h16), DSH // P),
            "cwqk": _part(np.concatenate([cwq[:, ds], cwk[:, ds]], axis=1).astype(h16), DKT),
            "cwv": _part(cwv[:, ds].astype(h16), DKT),
            "cwo": _part(cwo[ds, :].astype(h16), DSH // P),
            "wgu": _part(wgu_c, DKT),
            "wd": _part(wd_c, FFKT),
            "lmh": _part(lmh[:, vs].astype(h16), DKT),
            "cosT": cosT, "sinT": sinT, "rotM": rotM, "maskT": maskT,
        }
        in_maps.append(m)

    if "nc" not in _prog_cache:
        _prog_cache["nc"] = _build_program()
    nc = _prog_cache["nc"]

    res = run_bass_kernel_spmd(nc, in_maps, list(range(NCORES)))
    global LAST_RESULTS
    LAST_RESULTS = res
    logits = np.concatenate([r["logitsT"].T for r in res.results], axis=1)
    return logits.reshape(B, S, V).astype(np.float32)


if __name__ == "__main__":
    # quick build check
    nc = _build_program()
    print("program built ok")
